# revision 8
# baseline (speedup 1.0000x reference)
"""EGCNNet Trainium2 kernel: 8-core SPMD Bass implementation (self-contained)."""
import numpy as np

from concourse import bacc, bass, mybir, tile
from concourse.bass_utils import run_bass_kernel_spmd

N_NODES = 50000
N_EDGES = 800000
N_GRAPHS = 512
H = 64
F_NODE = 32
F_EDGE = 8
N_CLASSES = 10
EPS = 1e-5
N_CORES = 8
P = 128
BIAS16 = 32768
FP = mybir.dt.float32
BF = mybir.dt.bfloat16
AX = mybir.AxisListType
ALU = mybir.AluOpType
ACTF = mybir.ActivationFunctionType
RG = [list(range(N_CORES))]


# ---------------------------------------------------------------- host prep --
def _choose_buckets(hists):
    dmax = hists.shape[1] - 1
    cum = np.cumsum(hists[:, 1:], axis=1)

    def nodes(c, a, b):
        return cum[c, b - 1] - (cum[c, a - 2] if a >= 2 else 0)

    best = np.full(dmax + 1, np.inf)
    best[0] = 0.0
    choice = np.zeros(dmax + 1, np.int64)
    for b in range(1, dmax + 1):
        for a in range(1, b + 1):
            g = max(int(np.ceil(nodes(c, a, b) / P)) for c in range(N_CORES))
            cost = best[a - 1] + g * P * (int(np.ceil(b / 7.0)) * 8)
            if cost < best[b]:
                best[b] = cost
                choice[b] = a
    buckets = []
    b = dmax
    while b >= 1:
        a = int(choice[b])
        buckets.append((a, b))
        b = a - 1
    return buckets[::-1]


def prep(edge_index, batch):
    src = np.asarray(edge_index[0], np.int64)
    dst = np.asarray(edge_index[1], np.int64)
    batch = np.asarray(batch, np.int64)
    gpc = N_GRAPHS // N_CORES
    node_start = np.searchsorted(batch, np.arange(0, N_GRAPHS + 1, gpc))
    deg_global = np.bincount(dst, minlength=N_NODES).astype(np.int64)
    dmax = int(deg_global.max())
    hists = np.zeros((N_CORES, dmax + 1), np.int64)
    for c in range(N_CORES):
        lo, hi = int(node_start[c]), int(node_start[c + 1])
        hists[c] = np.bincount(deg_global[lo:hi], minlength=dmax + 1)
    buckets = _choose_buckets(hists)

    tmpl = []
    for (a, b) in buckets:
        g = max(int(np.ceil(hists[c, a:b + 1].sum() / P)) for c in range(N_CORES))
        if g > 0:
            tmpl.append((a, b, g))
    n_groups = sum(g for _, _, g in tmpl)
    nzero_max = max(int(hists[c, 0]) for c in range(N_CORES))
    zero_groups = int(np.ceil((nzero_max + 1) / P))
    n_pad = (n_groups + zero_groups) * P
    NT = N_CORES * n_pad
    TROWS = n_pad + NT
    assert TROWS <= 65536, f"table rows {TROWS} > 65536"
    ZR = n_groups * P

    # groups: (D, nb, slot0, c0) ; c0 in padded chunkcol units (8 per block)
    gmeta = []
    gbase = 0
    c0 = 0
    for (a, b, G) in tmpl:
        nb = int(np.ceil(b / 7.0))
        for g in range(G):
            gmeta.append((b, nb, (gbase + g) * P, c0))
            c0 += nb * 8
        gbase += G
    n_cc = c0  # padded chunkcols

    cores = []
    for c in range(N_CORES):
        lo, hi = int(node_start[c]), int(node_start[c + 1])
        nk = hi - lo
        deg = deg_global[lo:hi]
        em = (dst >= lo) & (dst < hi)
        e_ids = np.nonzero(em)[0]
        dl = dst[e_ids] - lo
        eo = np.lexsort((src[e_ids], dl))
        e_ids = e_ids[eo]
        dl = dl[eo]
        run_start = np.zeros(nk + 1, np.int64)
        np.cumsum(np.bincount(dl, minlength=nk), out=run_start[1:])

        old2slot = np.full(nk, -1, np.int64)
        gi = 0
        for (a, b, G) in tmpl:
            ids = np.nonzero((deg >= a) & (deg <= b))[0]
            for g in range(G):
                chunk = ids[g * P:(g + 1) * P]
                slot0 = gmeta[gi][2]
                old2slot[chunk] = slot0 + np.arange(len(chunk))
                gi += 1
        ids0 = np.nonzero(deg == 0)[0]
        old2slot[ids0] = ZR + 1 + np.arange(len(ids0))
        assert (old2slot >= 0).all()

        stream_eid = np.full(n_cc * P, -2, np.int64)  # -2 sentinel, -1 ZR-dummy
        slot_of = np.full(n_pad, -1, np.int64)
        slot_of[old2slot] = np.arange(nk)
        for (D, nb, slot0, cc0) in gmeta:
            nid = slot_of[slot0:slot0 + P]
            ok = nid >= 0
            d_n = np.where(ok, deg[np.maximum(nid, 0)], 0)
            for blk in range(nb):
                for jj in range(7):
                    j = blk * 7 + jj
                    cc = cc0 + blk * 8 + jj
                    if j >= D:
                        stream_eid[cc * P:(cc + 1) * P] = -1
                        continue
                    lane_ok = ok & (j < d_n)
                    e_sel = run_start[np.maximum(nid, 0)] + j
                    vals = np.where(lane_ok,
                                    e_ids[np.minimum(e_sel, max(len(e_ids) - 1, 0))],
                                    -1)
                    stream_eid[cc * P:(cc + 1) * P] = vals
                # 8th col stays -2 (sentinel)
        cores.append(dict(lo=lo, hi=hi, nk=nk, old2slot=old2slot,
                          stream_eid=stream_eid, deg=deg))

    old2new = np.zeros(N_NODES, np.int64)
    for c, ci in enumerate(cores):
        old2new[np.arange(ci["lo"], ci["hi"])] = c * n_pad + ci["old2slot"]

    for c, ci in enumerate(cores):
        se = ci["stream_eid"]
        gsrc = old2new[src[np.maximum(se, 0)]]
        own = gsrc // n_pad == c
        tpos = np.where(own, gsrc % n_pad, n_pad + gsrc)
        tpos = np.where(se >= 0, tpos, ZR)          # ZR-dummies & sentinel -> ZR
        tpos = np.where(se == -2, TROWS - 1, tpos)  # sentinel -> high row
        ci["tidx"] = tpos
        deg_slot = np.zeros(n_pad, np.float32)
        bat_slot = np.zeros(n_pad, np.float32)
        x_rows = np.full(n_pad, -1, np.int64)
        deg_slot[ci["old2slot"]] = ci["deg"]
        bat_slot[ci["old2slot"]] = (batch[ci["lo"]:ci["hi"]] - c * gpc)
        x_rows[ci["old2slot"]] = np.arange(ci["lo"], ci["hi"])
        ci["deg_slot"] = deg_slot
        ci["bat_slot"] = bat_slot
        ci["x_rows"] = x_rows

    return dict(cores=cores, tmpl=tmpl, gmeta=gmeta, n_pad=n_pad, NT=NT, ZR=ZR,
                n_cc=n_cc, node_start=node_start, TROWS=TROWS)


def wrap_idx16(idx):
    k = idx.shape[0]
    b = (idx - BIAS16).astype(np.int16)
    return np.tile(b.reshape(k // 16, 16).T, (8, 1)).copy()


# ------------------------------------------------------------ bass builder --
def build_nc(meta):
    n_pad, NT, ZR, n_cc, TROWS = (meta["n_pad"], meta["NT"], meta["ZR"],
                                  meta["n_cc"], meta["TROWS"])
    gmeta = meta["gmeta"]
    NCH_OWN = n_pad // P
    NCH_ALL = NT // P
    GPC = N_GRAPHS // N_CORES

    nc = bacc.Bacc("TRN2", target_bir_lowering=False, debug=False,
                   num_swdge_queues=2)

    def din(name, shape, dt=FP):
        return nc.declare_dram_parameter(name, list(shape), dt, isOutput=False)

    x_own = din("x_own", [NCH_OWN, F_NODE, P])
    x_all = din("x_all", [NCH_ALL, F_NODE, P])
    xT = din("xT", [F_NODE, NT])
    idxs = din("idxs", [P, n_cc * 8], mybir.dt.int16)
    eaT = din("eaT", [F_EDGE, n_cc * P])
    dinv_own = din("dinv_own", [P, NCH_OWN])
    dinv_all = din("dinv_all", [P, NCH_ALL])
    mask_own = din("mask_own", [P, NCH_OWN])
    mask_all = din("mask_all", [P, NCH_ALL])
    bat = din("bat", [P, NCH_OWN])
    w_l0 = din("w_l0", [3, F_NODE + 1, H])
    w_ly = din("w_ly", [3, 3, H + 1, H])
    w_ce0 = din("w_ce0", [F_EDGE, H])
    w_ce = din("w_ce", [3, H, H])
    w_hd = din("w_hd", [2, H + 1, H])
    bn_ly = din("bn_ly", [H, 8])   # cols: (w,b) x layers 1..3, pad
    bn_x = din("bn_x", [F_NODE, 2])
    bn_hd = din("bn_hd", [H, 4])
    out = nc.declare_dram_parameter("out", [N_GRAPHS, N_CLASSES], FP,
                                    isOutput=True)

    T1 = nc.dram_tensor("T1", [TROWS, 2 * H], FP)
    U1 = nc.dram_tensor("U1", [TROWS, H], FP)
    CeD = nc.dram_tensor("CeD", [3, P, n_cc, H], FP)
    ag_in = nc.dram_tensor("ag_in", [H, n_pad], BF)
    ag_out = nc.dram_tensor("ag_out", [N_CORES * H, n_pad], BF,
                            addr_space="Shared")
    hT_agD = nc.dram_tensor("hT_agD", [N_CORES, H, n_pad], BF)
    g_in = nc.dram_tensor("g_in", [H, GPC], FP)
    g_out = nc.dram_tensor("g_out", [N_CORES * H, GPC], FP, addr_space="Shared")

    with tile.TileContext(nc) as tc:
        with tc.tile_pool(name="persist", bufs=1) as pp, \
             tc.tile_pool(name="work", bufs=2) as wp, \
             tc.tile_pool(name="gath", bufs=2) as gp, \
             tc.tile_pool(name="ps", bufs=2, space="PSUM") as psp, \
             tc.tile_pool(name="pst", bufs=2, space="PSUM") as pst, \
             tc.tile_pool(name="psg", bufs=1, space="PSUM") as psg:

            it = pp.tile([P, n_cc * 8], mybir.dt.int16)
            nc.sync.dma_start(out=it[:], in_=idxs[:])
            hT_own = pp.tile([H, n_pad], FP)
            B_sb = pp.tile([P, NCH_OWN, H], FP)
            dinvo = pp.tile([P, NCH_OWN], FP)
            nc.sync.dma_start(out=dinvo[:], in_=dinv_own[:])
            dinva = pp.tile([P, NCH_ALL], FP)
            nc.sync.dma_start(out=dinva[:], in_=dinv_all[:])
            masko = pp.tile([P, NCH_OWN], FP)
            nc.sync.dma_start(out=masko[:], in_=mask_own[:])
            maska = pp.tile([P, NCH_ALL], FP)
            nc.sync.dma_start(out=maska[:], in_=mask_all[:])
            batv = pp.tile([P, NCH_OWN], FP)
            nc.sync.dma_start(out=batv[:], in_=bat[:])
            iota64 = pp.tile([P, H], mybir.dt.int32)
            nc.gpsimd.iota(iota64[:], pattern=[[1, H]], base=0,
                           channel_multiplier=0)
            iota64f = pp.tile([P, H], FP)
            nc.vector.tensor_copy(iota64f[:], iota64[:])
            idt = pp.tile([P, P], FP)
            from concourse.masks import make_identity
            make_identity(nc, idt[:])
            w0 = [pp.tile([F_NODE + 1, H], FP, name=f"w0_{k}", tag=f"w0_{k}") for k in range(3)]
            for k in range(3):
                nc.sync.dma_start(out=w0[k][:], in_=w_l0[k])
            wly = [[pp.tile([H + 1, H], FP, name=f"wly_{i}_{k}", tag=f"wly_{i}_{k}") for k in range(3)]
                   for i in range(3)]
            for i in range(3):
                for k in range(3):
                    nc.sync.dma_start(out=wly[i][k][:], in_=w_ly[i, k])
            wce0 = pp.tile([F_EDGE, H], FP)
            nc.sync.dma_start(out=wce0[:], in_=w_ce0[:])
            wce = [pp.tile([H, H], FP, name=f"wce_{k}", tag=f"wce_{k}") for k in range(3)]
            for k in range(3):
                nc.sync.dma_start(out=wce[k][:], in_=w_ce[k])
            whd = [pp.tile([H + 1, H], FP, name=f"whd_{k}", tag=f"whd_{k}") for k in range(2)]
            for k in range(2):
                nc.sync.dma_start(out=whd[k][:], in_=w_hd[k])
            bnly = pp.tile([H, 8], FP)
            nc.sync.dma_start(out=bnly[:], in_=bn_ly[:])
            bnx = pp.tile([F_NODE, 2], FP)
            nc.sync.dma_start(out=bnx[:], in_=bn_x[:])
            bnhd = pp.tile([H, 4], FP)
            nc.sync.dma_start(out=bnhd[:], in_=bn_hd[:])
            zrow = pp.tile([1, 2 * H], FP)
            nc.vector.memset(zrow[:], 0.0)
            epst = pp.tile([H, 1], FP)
            nc.vector.memset(epst[:], EPS)
            stats = pp.tile([H, 8], FP)
            lhs_a = pp.tile([H + 1, P], FP)
            lhs_b = pp.tile([H + 1, P], FP)
            nc.vector.memset(lhs_a[H:H + 1, :], 1.0)
            nc.vector.memset(lhs_b[H:H + 1, :], 1.0)
            nc.vector.memset(lhs_a[F_NODE:F_NODE + 1, :], 1.0)
            nc.vector.memset(lhs_b[F_NODE:F_NODE + 1, :], 1.0)
            nc.vector.memset(hT_own[:], 0.0)

            def bn_from_stats(nf, wb, cnt):
                """stats rows [:nf]: col0 sum, col1 sumsq -> col4 sw, col5 sh.
                wb: [nf, 2] AP with (w, b)."""
                s = stats[:nf, :]
                nc.vector.tensor_scalar(out=s[:, 2:3], in0=s[:, 0:1],
                                        scalar1=1.0 / cnt, scalar2=None,
                                        op0=ALU.mult)
                nc.vector.tensor_scalar(out=s[:, 3:4], in0=s[:, 1:2],
                                        scalar1=1.0 / cnt, scalar2=None,
                                        op0=ALU.mult)
                sq = wp.tile([H, 1], FP, tag="bnsq")
                nc.scalar.activation(out=sq[:nf, :], in_=s[:, 2:3],
                                     func=ACTF.Square)
                nc.vector.tensor_tensor(out=s[:, 3:4], in0=s[:, 3:4],
                                        in1=sq[:nf, :], op=ALU.subtract)
                nc.scalar.activation(out=s[:, 3:4], in_=s[:, 3:4],
                                     func=ACTF.Sqrt, bias=epst[:nf, :])
                nc.vector.reciprocal(out=s[:, 3:4], in_=s[:, 3:4])
                nc.vector.tensor_tensor(out=s[:, 4:5], in0=wb[:, 0:1],
                                        in1=s[:, 3:4], op=ALU.mult)
                nc.vector.tensor_tensor(out=s[:, 5:6], in0=s[:, 2:3],
                                        in1=s[:, 4:5], op=ALU.mult)
                nc.vector.tensor_tensor(out=s[:, 5:6], in0=wb[:, 1:2],
                                        in1=s[:, 5:6], op=ALU.subtract)

            def stats_from_hT_ag():
                acc = wp.tile([H, 2], FP, tag="sacc")
                for r in range(N_CORES):
                    slab = wp.tile([H, n_pad], BF, tag="slab")
                    nc.sync.dma_start(out=slab[:], in_=hT_agD[r])
                    t1 = wp.tile([H, 1], FP, tag="st1")
                    nc.vector.tensor_reduce(out=t1[:], in_=slab[:], axis=AX.X,
                                            op=ALU.add)
                    nc.vector.tensor_tensor(out=slab[:], in0=slab[:],
                                            in1=slab[:], op=ALU.mult)
                    t2 = wp.tile([H, 1], FP, tag="st2")
                    nc.vector.tensor_reduce(out=t2[:], in_=slab[:], axis=AX.X,
                                            op=ALU.add)
                    if r == 0:
                        nc.vector.tensor_copy(acc[:, 0:1], t1[:])
                        nc.vector.tensor_copy(acc[:, 1:2], t2[:])
                    else:
                        nc.vector.tensor_tensor(out=acc[:, 0:1], in0=acc[:, 0:1],
                                                in1=t1[:], op=ALU.add)
                        nc.vector.tensor_tensor(out=acc[:, 1:2], in0=acc[:, 1:2],
                                                in1=t2[:], op=ALU.add)
                nc.vector.tensor_copy(stats[:, 0:2], acc[:])

            def table_pass(li, own):
                """li=0: x->(h0|U1|U2); li>=1: h->(ht|A|B). own or full pass."""
                nch = NCH_OWN if own else NCH_ALL
                KD = F_NODE if li == 0 else H
                for ch in range(nch):
                    lhs = lhs_a if ch % 2 == 0 else lhs_b
                    if li == 0:
                        xt = wp.tile([F_NODE, P], FP, tag="xch")
                        nc.sync.dma_start(out=xt[:],
                                          in_=(x_own if own else x_all)[ch])
                        src_ap = xt[:]
                    else:
                        if own:
                            src_ap = hT_own[:, ch * P:(ch + 1) * P]
                        else:
                            slab = wp.tile([H, P], BF, tag="hslab")
                            nc.sync.dma_start(
                                out=slab[:],
                                in_=hT_agD[ch // NCH_OWN, :,
                                           (ch % NCH_OWN) * P:(ch % NCH_OWN + 1) * P])
                            src_ap = slab[:]
                    nc.vector.tensor_scalar(
                        out=lhs[:KD, :], in0=src_ap,
                        scalar1=stats[:KD, 4:5], scalar2=stats[:KD, 5:6],
                        op0=ALU.mult, op1=ALU.add)
                    ps = psp.tile([P, 192], FP, tag="tb")
                    if li == 0:
                        r0, r1, r2 = w0[0][:], w0[1][:], w0[2][:]
                    else:
                        r0, r1, r2 = (wly[li - 1][0][:], wly[li - 1][1][:],
                                      wly[li - 1][2][:])
                    nc.tensor.matmul(out=ps[:, 0:64], lhsT=lhs[:KD + 1, :],
                                     rhs=r0[:KD + 1, :], start=True, stop=True)
                    nc.tensor.matmul(out=ps[:, 64:128], lhsT=lhs[:KD + 1, :],
                                     rhs=r1[:KD + 1, :], start=True, stop=True)
                    nc.tensor.matmul(out=ps[:, 128:192], lhsT=lhs[:KD + 1, :],
                                     rhs=r2[:KD + 1, :], start=True, stop=True)
                    base = ch * P if own else n_pad + ch * P
                    if li == 0:
                        u1t = wp.tile([P, H], FP, tag="u1t")
                        nc.vector.tensor_copy(u1t[:], ps[:, 64:128])
                        nc.sync.dma_start(out=U1[base:base + P, :], in_=u1t[:])
                        if own:
                            nc.vector.tensor_copy(B_sb[:, ch, :], ps[:, 128:192])
                        h1t = wp.tile([P, H], FP, tag="h1t")
                        nc.scalar.activation(out=h1t[:], in_=ps[:, 0:64],
                                             func=ACTF.Relu)
                        nc.vector.tensor_scalar(
                            out=h1t[:], in0=h1t[:],
                            scalar1=(masko if own else maska)[:, ch:ch + 1],
                            scalar2=None, op0=ALU.mult)
                        tp = pst.tile([H, P], FP, tag="tpx")
                        nc.tensor.transpose(out=tp[:], in_=h1t[:], identity=idt[:])
                        if own:
                            nc.vector.tensor_copy(
                                hT_own[:, ch * P:(ch + 1) * P], tp[:])
                        else:
                            hbf = wp.tile([H, P], BF, tag="hbf")
                            nc.vector.tensor_copy(hbf[:], tp[:])
                            nc.sync.dma_start(
                                out=hT_agD[ch // NCH_OWN, :,
                                           (ch % NCH_OWN) * P:(ch % NCH_OWN + 1) * P],
                                in_=hbf[:])
                    else:
                        t1t = wp.tile([P, 2 * H], FP, tag="t1t")
                        nc.vector.tensor_scalar(
                            out=t1t[:, 0:64], in0=ps[:, 0:64],
                            scalar1=(dinvo if own else dinva)[:, ch:ch + 1],
                            scalar2=None, op0=ALU.mult)
                        nc.vector.tensor_copy(t1t[:, 64:128], ps[:, 64:128])
                        nc.sync.dma_start(out=T1[base:base + P, :], in_=t1t[:])
                        if own:
                            nc.vector.tensor_copy(B_sb[:, ch, :], ps[:, 128:192])

            # ================= layer 0 =================
            # x stats
            CW = 2048
            nxc = (NT + CW - 1) // CW
            for ci_ in range(nxc):
                w = min(CW, NT - ci_ * CW)
                xc = wp.tile([F_NODE, CW], FP, tag="xstat")
                nc.sync.dma_start(out=xc[:, :w], in_=xT[:, ci_ * CW:ci_ * CW + w])
                t1 = wp.tile([H, 1], FP, tag="st1")
                nc.vector.tensor_reduce(out=t1[:F_NODE, :], in_=xc[:, :w],
                                        axis=AX.X, op=ALU.add)
                nc.vector.tensor_tensor(out=xc[:, :w], in0=xc[:, :w],
                                        in1=xc[:, :w], op=ALU.mult)
                t2 = wp.tile([H, 1], FP, tag="st2")
                nc.vector.tensor_reduce(out=t2[:F_NODE, :], in_=xc[:, :w],
                                        axis=AX.X, op=ALU.add)
                if ci_ == 0:
                    nc.vector.tensor_copy(stats[:F_NODE, 0:1], t1[:F_NODE, :])
                    nc.vector.tensor_copy(stats[:F_NODE, 1:2], t2[:F_NODE, :])
                else:
                    nc.vector.tensor_tensor(out=stats[:F_NODE, 0:1],
                                            in0=stats[:F_NODE, 0:1],
                                            in1=t1[:F_NODE, :], op=ALU.add)
                    nc.vector.tensor_tensor(out=stats[:F_NODE, 1:2],
                                            in0=stats[:F_NODE, 1:2],
                                            in1=t2[:F_NODE, :], op=ALU.add)
            bn_from_stats(F_NODE, bnx[:], float(N_NODES))
            table_pass(0, True)
            table_pass(0, False)
            nc.sync.dma_start(out=U1[ZR:ZR + 1, :], in_=zrow[0:1, 0:H])

            # layer-0 edge phase: build er and Ce_i (per 8-col block)
            for (D, nb, slot0, cc0) in gmeta:
                gidx = slot0 // P
                for b_ in range(nb):
                    nreal = min(7, D - b_ * 7)
                    gtb = gp.tile([P, 8, H], FP, tag="gt0")
                    o = (cc0 + b_ * 8) * 8
                    nc.gpsimd.dma_gather(
                        out_ap=gtb[:], in_ap=U1[BIAS16:], idxs_ap=it[:, o:o + 64],
                        num_idxs=1024, num_idxs_reg=1024, elem_size=H,
                        queue_num=b_ % 2)
                    eab = wp.tile([F_EDGE, 8 * P], FP, tag="ea")
                    nc.sync.dma_start(
                        out=eab[:],
                        in_=eaT[:, (cc0 + b_ * 8) * P:(cc0 + b_ * 8 + 8) * P])
                    etb = wp.tile([P, 8, H], FP, tag="et")
                    for jj in range(nreal):
                        pse = pst.tile([P, H], FP, tag="tpx")
                        nc.tensor.matmul(out=pse[:],
                                         lhsT=eab[:, jj * P:(jj + 1) * P],
                                         rhs=wce0[:], start=True, stop=True)
                        nc.vector.tensor_tensor(out=etb[:, jj, :],
                                                in0=gtb[:, jj, :], in1=pse[:],
                                                op=ALU.add)
                    nc.vector.tensor_tensor(
                        out=etb[:, 0:nreal, :], in0=etb[:, 0:nreal, :],
                        in1=B_sb[:, gidx:gidx + 1, :].to_broadcast([P, nreal, H]),
                        op=ALU.add)
                    nc.scalar.activation(out=etb[:, 0:nreal, :],
                                         in_=etb[:, 0:nreal, :], func=ACTF.Relu)
                    ce3b = wp.tile([P, 8, 192], FP, tag="ce3")
                    for jj in range(nreal):
                        tp = pst.tile([H, P], FP, tag="tpx")
                        nc.tensor.transpose(out=tp[:], in_=etb[:, jj, :],
                                            identity=idt[:])
                        erT = wp.tile([H, P], FP, tag="erT")
                        nc.vector.tensor_copy(erT[:], tp[:])
                        psc = psp.tile([P, 192], FP, tag="tb")
                        for i3 in range(3):
                            nc.tensor.matmul(out=psc[:, i3 * 64:(i3 + 1) * 64],
                                             lhsT=erT[:], rhs=wce[i3][:],
                                             start=True, stop=True)
                        nc.vector.tensor_copy(ce3b[:, jj, :], psc[:])
                    for i3 in range(3):
                        nc.sync.dma_start(
                            out=CeD[i3, :, cc0 + b_ * 8:cc0 + b_ * 8 + 8, :],
                            in_=ce3b[:, :, i3 * 64:(i3 + 1) * 64])

            # h1 stats (hT_agD was filled during full pass)
            stats_from_hT_ag()

            # ================= layers 1..3 =================
            gps = psg.tile([H, H], FP, name="gps", tag="hd", padded_shape=[H, 512])   # pooling accum
            for li in range(1, 4):
                bn_from_stats(H, bnly[:, (li - 1) * 2:(li - 1) * 2 + 2],
                              float(N_NODES))
                table_pass(li, True)
                table_pass(li, False)
                nc.sync.dma_start(out=T1[ZR:ZR + 1, :], in_=zrow[0:1, :])

                for gi_, (D, nb, slot0, cc0) in enumerate(gmeta):
                    gidx = slot0 // P
                    hacc = wp.tile([P, H], FP, tag="hacc")
                    for b_ in range(nb):
                        nreal = min(7, D - b_ * 7)
                        gtb = gp.tile([P, 8, 2 * H], FP, tag="gt")
                        o = (cc0 + b_ * 8) * 8
                        nc.gpsimd.dma_gather(
                            out_ap=gtb[:], in_ap=T1[BIAS16:],
                            idxs_ap=it[:, o:o + 64],
                            num_idxs=1024, num_idxs_reg=1024, elem_size=2 * H,
                            queue_num=b_ % 2)
                        ceb = gp.tile([P, 8, H], FP, tag="cet")
                        nc.sync.dma_start(
                            out=ceb[:],
                            in_=CeD[li - 1, :, cc0 + b_ * 8:cc0 + b_ * 8 + 8, :])
                        nc.vector.tensor_tensor(
                            out=ceb[:, 0:nreal, :], in0=ceb[:, 0:nreal, :],
                            in1=gtb[:, 0:nreal, H:2 * H], op=ALU.add)
                        nc.vector.tensor_tensor(
                            out=ceb[:, 0:nreal, :], in0=ceb[:, 0:nreal, :],
                            in1=B_sb[:, gidx:gidx + 1, :].to_broadcast(
                                [P, nreal, H]),
                            op=ALU.add)
                        nc.scalar.activation(out=ceb[:, 0:nreal, :],
                                             in_=ceb[:, 0:nreal, :],
                                             func=ACTF.Sigmoid)
                        nc.vector.tensor_tensor(
                            out=ceb[:, 0:nreal, :], in0=ceb[:, 0:nreal, :],
                            in1=gtb[:, 0:nreal, 0:H], op=ALU.mult)
                        rr = wp.tile([P, H], FP, tag="rr")
                        nc.vector.tensor_reduce(
                            out=rr[:],
                            in_=ceb[:, 0:nreal, :].transpose([0, 2, 1]),
                            axis=AX.X, op=ALU.add)
                        if b_ == 0:
                            nc.vector.tensor_copy(hacc[:], rr[:])
                        else:
                            nc.vector.tensor_tensor(out=hacc[:], in0=hacc[:],
                                                    in1=rr[:], op=ALU.add)
                    nc.scalar.activation(out=hacc[:], in_=hacc[:], func=ACTF.Relu)
                    nc.vector.tensor_scalar(out=hacc[:], in0=hacc[:],
                                            scalar1=dinvo[:, gidx:gidx + 1],
                                            scalar2=None, op0=ALU.mult)
                    if li == 3:
                        gpm = wp.tile([P, H], FP, tag="gpm")
                        nc.vector.tensor_scalar(out=gpm[:], in0=iota64f[:],
                                                scalar1=batv[:, gidx:gidx + 1],
                                                scalar2=None, op0=ALU.is_equal)
                        nc.tensor.matmul(out=gps[:], lhsT=hacc[:], rhs=gpm[:],
                                         start=(gi_ == 0),
                                         stop=(gi_ == len(gmeta) - 1),
                                         skip_group_check=True)
                    else:
                        tp = pst.tile([H, P], FP, tag="tpx")
                        nc.tensor.transpose(out=tp[:], in_=hacc[:],
                                            identity=idt[:])
                        nc.vector.tensor_copy(hT_own[:, slot0:slot0 + P], tp[:])

                if li < 3:
                    hbf = wp.tile([H, n_pad], BF, tag="agbf")
                    nc.vector.tensor_copy(hbf[:], hT_own[:])
                    nc.sync.dma_start(out=ag_in[:], in_=hbf[:])
                    nc.gpsimd.collective_compute(
                        "AllGather", ALU.bypass, replica_groups=RG,
                        ins=[ag_in[:]], outs=[ag_out[:]])
                    for r in range(N_CORES):
                        nc.sync.dma_start(out=hT_agD[r],
                                          in_=ag_out[r * H:(r + 1) * H, :])
                    stats_from_hT_ag()

            # ================= head =================
            gsb = wp.tile([H, H], FP, tag="gsb")
            nc.vector.tensor_copy(gsb[:], gps[:])
            nc.sync.dma_start(out=g_in[:], in_=gsb[:, 0:GPC])
            nc.gpsimd.collective_compute(
                "AllGather", ALU.bypass, replica_groups=RG,
                ins=[g_in[:]], outs=[g_out[:]])
            gT = pp.tile([H + 1, N_GRAPHS], FP)
            nc.vector.memset(gT[H:H + 1, :], 1.0)
            for r in range(N_CORES):
                nc.sync.dma_start(out=gT[:H, r * GPC:(r + 1) * GPC],
                                  in_=g_out[r * H:(r + 1) * H, :])
            # BN over 512 graphs
            nc.vector.tensor_reduce(out=stats[:, 0:1], in_=gT[:H, :], axis=AX.X,
                                    op=ALU.add)
            sq5 = wp.tile([H, N_GRAPHS], FP, tag="sq5")
            nc.scalar.activation(out=sq5[:], in_=gT[:H, :], func=ACTF.Square)
            nc.vector.tensor_reduce(out=stats[:, 1:2], in_=sq5[:], axis=AX.X,
                                    op=ALU.add)
            bn_from_stats(H, bnhd[:, 0:2], float(N_GRAPHS))
            gbn = pp.tile([H + 1, N_GRAPHS], FP)
            nc.vector.memset(gbn[H:H + 1, :], 1.0)
            nc.vector.tensor_scalar(out=gbn[:H, :], in0=gT[:H, :],
                                    scalar1=stats[:, 4:5], scalar2=stats[:, 5:6],
                                    op0=ALU.mult, op1=ALU.add)
            ph = psg.tile([H, N_GRAPHS], FP, name="ph", tag="hd")
            nc.tensor.matmul(out=ph[:], lhsT=whd[0][:], rhs=gbn[:], start=True,
                             stop=True)
            nc.scalar.activation(out=gT[:H, :], in_=ph[:], func=ACTF.Relu)
            nc.vector.tensor_reduce(out=stats[:, 0:1], in_=gT[:H, :], axis=AX.X,
                                    op=ALU.add)
            nc.scalar.activation(out=sq5[:], in_=gT[:H, :], func=ACTF.Square)
            nc.vector.tensor_reduce(out=stats[:, 1:2], in_=sq5[:], axis=AX.X,
                                    op=ALU.add)
            bn_from_stats(H, bnhd[:, 2:4], float(N_GRAPHS))
            nc.vector.tensor_scalar(out=gbn[:H, :], in0=gT[:H, :],
                                    scalar1=stats[:, 4:5], scalar2=stats[:, 5:6],
                                    op0=ALU.mult, op1=ALU.add)
            pl = psg.tile([H, N_GRAPHS], FP, name="pl", tag="hd")
            nc.tensor.matmul(out=pl[:], lhsT=whd[1][:], rhs=gbn[:], start=True,
                             stop=True)
            lgT = pp.tile([H, N_GRAPHS], FP)
            nc.vector.tensor_copy(lgT[:], pl[:])
            # log_softmax per graph: transpose chunks of 128 graphs
            for cg in range(N_GRAPHS // P):
                tp = pst.tile([P, P], FP, tag="tpx")
                nc.tensor.transpose(out=tp[:, 0:N_CLASSES],
                                    in_=lgT[0:N_CLASSES, cg * P:(cg + 1) * P],
                                    identity=idt[0:N_CLASSES, 0:N_CLASSES])
                lg = wp.tile([P, N_CLASSES], FP, tag="lg")
                nc.vector.tensor_copy(lg[:], tp[:, 0:N_CLASSES])
                mx = wp.tile([P, 1], FP, tag="mx")
                nc.vector.tensor_reduce(out=mx[:], in_=lg[:], axis=AX.X,
                                        op=ALU.max)
                nc.vector.tensor_scalar(out=lg[:], in0=lg[:], scalar1=mx[:],
                                        scalar2=None, op0=ALU.subtract)
                ex = wp.tile([P, N_CLASSES], FP, tag="ex")
                sm = wp.tile([P, 1], FP, tag="sm")
                nc.scalar.activation(out=ex[:], in_=lg[:], func=ACTF.Exp,
                                     accum_out=sm[:])
                nc.scalar.activation(out=sm[:], in_=sm[:], func=ACTF.Ln)
                nc.vector.tensor_scalar(out=lg[:], in0=lg[:], scalar1=sm[:],
                                        scalar2=None, op0=ALU.subtract)
                nc.sync.dma_start(out=out[cg * P:(cg + 1) * P, :], in_=lg[:])

    nc.finalize()
    return nc


# --------------------------------------------------------------- run + host --
_CACHE = {}


def kernel(**inputs):
    inp = {k: np.asarray(v) for k, v in inputs.items()}
    meta = prep(inp["edge_index"], inp["batch"])
    n_pad, NT, ZR = meta["n_pad"], meta["NT"], meta["ZR"]
    NCH_OWN, NCH_ALL = n_pad // P, NT // P
    n_cc = meta["n_cc"]

    key = (n_pad, NT, n_cc, tuple(meta["tmpl"]))
    if key not in _CACHE:
        _CACHE[key] = build_nc(meta)
    nc = _CACHE[key]

    # global padded arrays
    x_pad = np.zeros((NT, F_NODE), np.float32)
    deg_pad = np.zeros(NT, np.float32)
    mask_pad = np.zeros(NT, np.float32)
    for c, ci in enumerate(meta["cores"]):
        b0 = c * n_pad
        sel = ci["x_rows"] >= 0
        ii = np.nonzero(sel)[0]
        x_pad[b0 + ii] = inp["x"][ci["x_rows"][ii]]
        deg_pad[b0:b0 + n_pad] = ci["deg_slot"]
        mask_pad[b0 + ii] = 1.0
    dinv_pad = (1.0 / np.sqrt(np.maximum(deg_pad, 1.0))).astype(np.float32)

    def chunked(a, nch):  # [nch*128, F] -> [nch, F, 128]
        return np.ascontiguousarray(
            a.reshape(nch, P, -1).transpose(0, 2, 1)).astype(np.float32)

    def cols(a, nch):  # [nch*128] -> [128, nch]
        return np.ascontiguousarray(a.reshape(nch, P).T).astype(np.float32)

    x_all = chunked(x_pad, NCH_ALL)
    xT = np.ascontiguousarray(x_pad.T).astype(np.float32)
    dinv_all = cols(dinv_pad, NCH_ALL)
    mask_all = cols(mask_pad, NCH_ALL)

    # weights
    Wn0, bn0 = inp["Wn0"], inp["bn0"]
    We0 = inp["We0"]
    w_l0 = np.stack([
        np.vstack([Wn0, bn0[None, :]]),
        np.vstack([Wn0 @ We0[:H], (bn0 @ We0[:H])[None, :]]),
        np.vstack([Wn0 @ We0[H:2 * H],
                   (bn0 @ We0[H:2 * H] + inp["be0"])[None, :]]),
    ]).astype(np.float32)
    w_ly = np.zeros((3, 3, H + 1, H), np.float32)
    for i in range(3):
        Wn, Wnb, We, Web = inp["Wn"][i], inp["Wnb"][i], inp["We"][i], inp["Web"][i]
        w_ly[i, 0] = np.vstack([Wn, Wnb[None, :]])
        w_ly[i, 1] = np.vstack([Wn @ We[:H], (Wnb @ We[:H])[None, :]])
        w_ly[i, 2] = np.vstack([Wn @ We[H:2 * H],
                                (Wnb @ We[H:2 * H] + Web)[None, :]])
    w_ce0 = We0[2 * H:].astype(np.float32)
    w_ce = np.stack([inp["We"][i, 2 * H:] for i in range(3)]).astype(np.float32)
    w_hd = np.zeros((2, H + 1, H), np.float32)
    w_hd[0] = np.vstack([inp["Wfc"], inp["bfc"][None, :]])
    w_hd[1, :H, :N_CLASSES] = inp["Wcls"]
    w_hd[1, H, :N_CLASSES] = inp["bcls"]
    bn_ly = np.zeros((H, 8), np.float32)
    for i in range(3):
        bn_ly[:, 2 * i] = inp["bns_w"][i]
        bn_ly[:, 2 * i + 1] = inp["bns_b"][i]
    bn_x = np.stack([inp["bn_feat_w"], inp["bn_feat_b"]], 1).astype(np.float32)
    bn_hd = np.stack([inp["bn_fc_w"], inp["bn_fc_b"],
                      inp["bn_hid_w"], inp["bn_hid_b"]], 1).astype(np.float32)

    in_maps = []
    for c, ci in enumerate(meta["cores"]):
        b0 = c * n_pad
        x_own_pad = x_pad[b0:b0 + n_pad]
        ea_s = np.zeros((n_cc * P, F_EDGE), np.float32)
        se = ci["stream_eid"]
        v = se >= 0
        ea_s[v] = inp["edge_attr"][se[v]]
        in_maps.append(dict(
            x_own=chunked(x_own_pad, NCH_OWN),
            x_all=x_all, xT=xT,
            idxs=wrap_idx16(ci["tidx"]),
            eaT=np.ascontiguousarray(ea_s.T).astype(np.float32),
            dinv_own=cols(dinv_pad[b0:b0 + n_pad], NCH_OWN),
            dinv_all=dinv_all,
            mask_own=cols(mask_pad[b0:b0 + n_pad], NCH_OWN),
            mask_all=mask_all,
            bat=cols(ci["bat_slot"], NCH_OWN),
            w_l0=w_l0, w_ly=w_ly, w_ce0=w_ce0, w_ce=w_ce, w_hd=w_hd,
            bn_ly=bn_ly, bn_x=bn_x, bn_hd=bn_hd,
        ))

    res = run_bass_kernel_spmd(nc, in_maps, list(range(N_CORES)))
    return np.asarray(res.results[0]["out"], np.float32)


# revision 11
# speedup vs baseline: 107.4397x; 107.4397x over previous
"""EGCNNet Trainium2 kernel: 8-core SPMD Bass implementation (self-contained)."""
import numpy as np

from concourse import bacc, bass, mybir, tile
from concourse.bass_utils import run_bass_kernel_spmd

N_NODES = 50000
N_EDGES = 800000
N_GRAPHS = 512
H = 64
F_NODE = 32
F_EDGE = 8
N_CLASSES = 10
EPS = 1e-5
N_CORES = 8
P = 128
BIAS16 = 32768
FP = mybir.dt.float32
BF = mybir.dt.bfloat16
AX = mybir.AxisListType
ALU = mybir.AluOpType
ACTF = mybir.ActivationFunctionType
RG = [list(range(N_CORES))]


# ---------------------------------------------------------------- host prep --
def _choose_buckets(hists):
    dmax = hists.shape[1] - 1
    cum = np.cumsum(hists[:, 1:], axis=1)

    def nodes(c, a, b):
        return cum[c, b - 1] - (cum[c, a - 2] if a >= 2 else 0)

    best = np.full(dmax + 1, np.inf)
    best[0] = 0.0
    choice = np.zeros(dmax + 1, np.int64)
    for b in range(1, dmax + 1):
        for a in range(1, b + 1):
            g = max(int(np.ceil(nodes(c, a, b) / P)) for c in range(N_CORES))
            cost = best[a - 1] + g * P * (int(np.ceil(b / 7.0)) * 8)
            if cost < best[b]:
                best[b] = cost
                choice[b] = a
    buckets = []
    b = dmax
    while b >= 1:
        a = int(choice[b])
        buckets.append((a, b))
        b = a - 1
    return buckets[::-1]


def prep(edge_index, batch):
    src = np.asarray(edge_index[0], np.int64)
    dst = np.asarray(edge_index[1], np.int64)
    batch = np.asarray(batch, np.int64)
    gpc = N_GRAPHS // N_CORES
    node_start = np.searchsorted(batch, np.arange(0, N_GRAPHS + 1, gpc))
    deg_global = np.bincount(dst, minlength=N_NODES).astype(np.int64)
    dmax = int(deg_global.max())
    hists = np.zeros((N_CORES, dmax + 1), np.int64)
    for c in range(N_CORES):
        lo, hi = int(node_start[c]), int(node_start[c + 1])
        hists[c] = np.bincount(deg_global[lo:hi], minlength=dmax + 1)
    buckets = _choose_buckets(hists)

    tmpl = []
    for (a, b) in buckets:
        g = max(int(np.ceil(hists[c, a:b + 1].sum() / P)) for c in range(N_CORES))
        if g > 0:
            tmpl.append((a, b, g))
    n_groups = sum(g for _, _, g in tmpl)
    nzero_max = max(int(hists[c, 0]) for c in range(N_CORES))
    zero_groups = int(np.ceil((nzero_max + 1) / P))
    n_pad = (n_groups + zero_groups) * P
    NT = N_CORES * n_pad
    TROWS = n_pad + NT
    assert TROWS <= 65536, f"table rows {TROWS} > 65536"
    ZR = n_groups * P

    # groups: (D, nb, slot0, c0) ; c0 in padded chunkcol units (8 per block)
    gmeta = []
    gbase = 0
    c0 = 0
    for (a, b, G) in tmpl:
        nb = int(np.ceil(b / 7.0))
        for g in range(G):
            gmeta.append((b, nb, (gbase + g) * P, c0))
            c0 += nb * 8
        gbase += G
    n_cc = c0  # padded chunkcols

    cores = []
    for c in range(N_CORES):
        lo, hi = int(node_start[c]), int(node_start[c + 1])
        nk = hi - lo
        deg = deg_global[lo:hi]
        em = (dst >= lo) & (dst < hi)
        e_ids = np.nonzero(em)[0]
        dl = dst[e_ids] - lo
        eo = np.lexsort((src[e_ids], dl))
        e_ids = e_ids[eo]
        dl = dl[eo]
        run_start = np.zeros(nk + 1, np.int64)
        np.cumsum(np.bincount(dl, minlength=nk), out=run_start[1:])

        old2slot = np.full(nk, -1, np.int64)
        gi = 0
        for (a, b, G) in tmpl:
            ids = np.nonzero((deg >= a) & (deg <= b))[0]
            for g in range(G):
                chunk = ids[g * P:(g + 1) * P]
                slot0 = gmeta[gi][2]
                old2slot[chunk] = slot0 + np.arange(len(chunk))
                gi += 1
        ids0 = np.nonzero(deg == 0)[0]
        old2slot[ids0] = ZR + 1 + np.arange(len(ids0))
        assert (old2slot >= 0).all()

        stream_eid = np.full(n_cc * P, -2, np.int64)  # -2 sentinel, -1 ZR-dummy
        slot_of = np.full(n_pad, -1, np.int64)
        slot_of[old2slot] = np.arange(nk)
        for (D, nb, slot0, cc0) in gmeta:
            nid = slot_of[slot0:slot0 + P]
            ok = nid >= 0
            d_n = np.where(ok, deg[np.maximum(nid, 0)], 0)
            for blk in range(nb):
                for jj in range(7):
                    j = blk * 7 + jj
                    cc = cc0 + blk * 8 + jj
                    if j >= D:
                        stream_eid[cc * P:(cc + 1) * P] = -1
                        continue
                    lane_ok = ok & (j < d_n)
                    e_sel = run_start[np.maximum(nid, 0)] + j
                    vals = np.where(lane_ok,
                                    e_ids[np.minimum(e_sel, max(len(e_ids) - 1, 0))],
                                    -1)
                    stream_eid[cc * P:(cc + 1) * P] = vals
                # 8th col stays -2 (sentinel)
        cores.append(dict(lo=lo, hi=hi, nk=nk, old2slot=old2slot,
                          stream_eid=stream_eid, deg=deg))

    old2new = np.zeros(N_NODES, np.int64)
    for c, ci in enumerate(cores):
        old2new[np.arange(ci["lo"], ci["hi"])] = c * n_pad + ci["old2slot"]

    for c, ci in enumerate(cores):
        se = ci["stream_eid"]
        gsrc = old2new[src[np.maximum(se, 0)]]
        own = gsrc // n_pad == c
        tpos = np.where(own, gsrc % n_pad, n_pad + gsrc)
        tpos = np.where(se >= 0, tpos, ZR)          # ZR-dummies & sentinel -> ZR
        tpos = np.where(se == -2, TROWS - 1, tpos)  # sentinel -> high row
        ci["tidx"] = tpos
        deg_slot = np.zeros(n_pad, np.float32)
        bat_slot = np.zeros(n_pad, np.float32)
        x_rows = np.full(n_pad, -1, np.int64)
        deg_slot[ci["old2slot"]] = ci["deg"]
        bat_slot[ci["old2slot"]] = (batch[ci["lo"]:ci["hi"]] - c * gpc)
        x_rows[ci["old2slot"]] = np.arange(ci["lo"], ci["hi"])
        ci["deg_slot"] = deg_slot
        ci["bat_slot"] = bat_slot
        ci["x_rows"] = x_rows

    return dict(cores=cores, tmpl=tmpl, gmeta=gmeta, n_pad=n_pad, NT=NT, ZR=ZR,
                n_cc=n_cc, node_start=node_start, TROWS=TROWS)


def wrap_idx16(idx):
    k = idx.shape[0]
    b = (idx - BIAS16).astype(np.int16)
    return np.tile(b.reshape(k // 16, 16).T, (8, 1)).copy()


# ------------------------------------------------------------ bass builder --
def build_nc(meta):
    n_pad, NT, ZR, n_cc, TROWS = (meta["n_pad"], meta["NT"], meta["ZR"],
                                  meta["n_cc"], meta["TROWS"])
    gmeta = meta["gmeta"]
    NCH_OWN = n_pad // P
    NCH_ALL = NT // P
    GPC = N_GRAPHS // N_CORES

    nc = bacc.Bacc("TRN2", target_bir_lowering=False, debug=False,
                   num_swdge_queues=2)

    def din(name, shape, dt=FP):
        return nc.declare_dram_parameter(name, list(shape), dt, isOutput=False)

    xT_own = din("xT_own", [F_NODE, n_pad])
    xT = din("xT", [F_NODE, NT])
    idxs = din("idxs", [P, n_cc * 8], mybir.dt.int16)
    eaT = din("eaT", [F_EDGE, n_cc * P])
    dinv_own = din("dinv_own", [P, NCH_OWN])
    dinv_all = din("dinv_all", [P, NCH_ALL])
    mask_own = din("mask_own", [P, NCH_OWN])
    mask_all = din("mask_all", [P, NCH_ALL])
    bat = din("bat", [P, NCH_OWN])
    w_l0 = din("w_l0", [3, F_NODE + 1, H])
    w_ly = din("w_ly", [3, 3, H + 1, H])
    w_ce0 = din("w_ce0", [F_EDGE, H])
    w_ce = din("w_ce", [3, H, H])
    w_hd = din("w_hd", [2, H + 1, H])
    bn_ly = din("bn_ly", [H, 8])   # cols: (w,b) x layers 1..3, pad
    bn_x = din("bn_x", [F_NODE, 2])
    bn_hd = din("bn_hd", [H, 4])
    out = nc.declare_dram_parameter("out", [N_GRAPHS, N_CLASSES], FP,
                                    isOutput=True)

    T1 = nc.dram_tensor("T1", [TROWS, 2 * H], FP)
    U1 = nc.dram_tensor("U1", [TROWS, H], FP)
    CeD = nc.dram_tensor("CeD", [3, P, n_cc, H], FP)
    ag_in = nc.dram_tensor("ag_in", [H, n_pad], BF)
    ag_out = nc.dram_tensor("ag_out", [N_CORES * H, n_pad], BF,
                            addr_space="Shared")
    hT_agD = nc.dram_tensor("hT_agD", [N_CORES, H, n_pad], BF)
    g_in = nc.dram_tensor("g_in", [H, GPC], FP)
    g_out = nc.dram_tensor("g_out", [N_CORES * H, GPC], FP, addr_space="Shared")

    with tile.TileContext(nc) as tc:
        with tc.tile_pool(name="persist", bufs=1) as pp, \
             tc.tile_pool(name="work", bufs=2) as wp, \
             tc.tile_pool(name="gath", bufs=2) as gp, \
             tc.tile_pool(name="ps", bufs=2, space="PSUM") as psp, \
             tc.tile_pool(name="pst", bufs=2, space="PSUM") as pst, \
             tc.tile_pool(name="psg", bufs=1, space="PSUM") as psg:

            it = pp.tile([P, n_cc * 8], mybir.dt.int16)
            nc.sync.dma_start(out=it[:], in_=idxs[:])
            hT_own = pp.tile([H, n_pad], FP)
            B_sb = pp.tile([P, NCH_OWN, H], FP)
            dinvo = pp.tile([P, NCH_OWN], FP)
            nc.sync.dma_start(out=dinvo[:], in_=dinv_own[:])
            dinva = pp.tile([P, NCH_ALL], FP)
            nc.sync.dma_start(out=dinva[:], in_=dinv_all[:])
            masko = pp.tile([P, NCH_OWN], FP)
            nc.sync.dma_start(out=masko[:], in_=mask_own[:])
            maska = pp.tile([P, NCH_ALL], FP)
            nc.sync.dma_start(out=maska[:], in_=mask_all[:])
            batv = pp.tile([P, NCH_OWN], FP)
            nc.sync.dma_start(out=batv[:], in_=bat[:])
            iota64 = pp.tile([P, H], mybir.dt.int32)
            nc.gpsimd.iota(iota64[:], pattern=[[1, H]], base=0,
                           channel_multiplier=0)
            iota64f = pp.tile([P, H], FP)
            nc.vector.tensor_copy(iota64f[:], iota64[:])
            idt = pp.tile([P, P], FP)
            from concourse.masks import make_identity
            make_identity(nc, idt[:])
            w0 = [pp.tile([F_NODE + 1, H], FP, name=f"w0_{k}", tag=f"w0_{k}") for k in range(3)]
            for k in range(3):
                nc.sync.dma_start(out=w0[k][:], in_=w_l0[k])
            wly = [[pp.tile([H + 1, H], FP, name=f"wly_{i}_{k}", tag=f"wly_{i}_{k}") for k in range(3)]
                   for i in range(3)]
            for i in range(3):
                for k in range(3):
                    nc.sync.dma_start(out=wly[i][k][:], in_=w_ly[i, k])
            wce0 = pp.tile([F_EDGE, H], FP)
            nc.sync.dma_start(out=wce0[:], in_=w_ce0[:])
            wce = [pp.tile([H, H], FP, name=f"wce_{k}", tag=f"wce_{k}") for k in range(3)]
            for k in range(3):
                nc.sync.dma_start(out=wce[k][:], in_=w_ce[k])
            whd = [pp.tile([H + 1, H], FP, name=f"whd_{k}", tag=f"whd_{k}") for k in range(2)]
            for k in range(2):
                nc.sync.dma_start(out=whd[k][:], in_=w_hd[k])
            bnly = pp.tile([H, 8], FP)
            nc.sync.dma_start(out=bnly[:], in_=bn_ly[:])
            bnx = pp.tile([F_NODE, 2], FP)
            nc.sync.dma_start(out=bnx[:], in_=bn_x[:])
            bnhd = pp.tile([H, 4], FP)
            nc.sync.dma_start(out=bnhd[:], in_=bn_hd[:])
            zrow = pp.tile([1, 2 * H], FP)
            nc.vector.memset(zrow[:], 0.0)
            epst = pp.tile([H, 1], FP)
            nc.vector.memset(epst[:], EPS)
            stats = pp.tile([H, 8], FP)
            lhs_a = pp.tile([H + 1, P], FP)
            lhs_b = pp.tile([H + 1, P], FP)
            nc.vector.memset(lhs_a[H:H + 1, :], 1.0)
            nc.vector.memset(lhs_b[H:H + 1, :], 1.0)
            nc.vector.memset(lhs_a[F_NODE:F_NODE + 1, :], 1.0)
            nc.vector.memset(lhs_b[F_NODE:F_NODE + 1, :], 1.0)
            nc.vector.memset(hT_own[:], 0.0)

            def bn_from_stats(nf, wb, cnt):
                """stats rows [:nf]: col0 sum, col1 sumsq -> col4 sw, col5 sh.
                wb: [nf, 2] AP with (w, b)."""
                s = stats[:nf, :]
                nc.vector.tensor_scalar(out=s[:, 2:3], in0=s[:, 0:1],
                                        scalar1=1.0 / cnt, scalar2=None,
                                        op0=ALU.mult)
                nc.vector.tensor_scalar(out=s[:, 3:4], in0=s[:, 1:2],
                                        scalar1=1.0 / cnt, scalar2=None,
                                        op0=ALU.mult)
                sq = wp.tile([H, 1], FP, tag="bnsq")
                nc.scalar.activation(out=sq[:nf, :], in_=s[:, 2:3],
                                     func=ACTF.Square)
                nc.vector.tensor_tensor(out=s[:, 3:4], in0=s[:, 3:4],
                                        in1=sq[:nf, :], op=ALU.subtract)
                nc.scalar.activation(out=s[:, 3:4], in_=s[:, 3:4],
                                     func=ACTF.Sqrt, bias=epst[:nf, :])
                nc.vector.reciprocal(out=s[:, 3:4], in_=s[:, 3:4])
                nc.vector.tensor_tensor(out=s[:, 4:5], in0=wb[:, 0:1],
                                        in1=s[:, 3:4], op=ALU.mult)
                nc.vector.tensor_tensor(out=s[:, 5:6], in0=s[:, 2:3],
                                        in1=s[:, 4:5], op=ALU.mult)
                nc.vector.tensor_tensor(out=s[:, 5:6], in0=wb[:, 1:2],
                                        in1=s[:, 5:6], op=ALU.subtract)

            def stats_from_hT_ag():
                acc = wp.tile([H, 2], FP, tag="sacc")
                for r in range(N_CORES):
                    slab = wp.tile([H, n_pad], BF, tag="slab")
                    nc.sync.dma_start(out=slab[:], in_=hT_agD[r])
                    t1 = wp.tile([H, 1], FP, tag="st1")
                    nc.vector.tensor_reduce(out=t1[:], in_=slab[:], axis=AX.X,
                                            op=ALU.add)
                    nc.vector.tensor_tensor(out=slab[:], in0=slab[:],
                                            in1=slab[:], op=ALU.mult)
                    t2 = wp.tile([H, 1], FP, tag="st2")
                    nc.vector.tensor_reduce(out=t2[:], in_=slab[:], axis=AX.X,
                                            op=ALU.add)
                    if r == 0:
                        nc.vector.tensor_copy(acc[:, 0:1], t1[:])
                        nc.vector.tensor_copy(acc[:, 1:2], t2[:])
                    else:
                        nc.vector.tensor_tensor(out=acc[:, 0:1], in0=acc[:, 0:1],
                                                in1=t1[:], op=ALU.add)
                        nc.vector.tensor_tensor(out=acc[:, 1:2], in0=acc[:, 1:2],
                                                in1=t2[:], op=ALU.add)
                nc.vector.tensor_copy(stats[:, 0:2], acc[:])

            def table_pass(li, own):
                """li=0: x->(h0|U1|U2); li>=1: h->(ht|A|B). own or full pass."""
                nch = NCH_OWN if own else NCH_ALL
                KD = F_NODE if li == 0 else H
                for ch in range(nch):
                    lhs = lhs_a if ch % 2 == 0 else lhs_b
                    if li == 0:
                        xt = wp.tile([F_NODE, P], FP, tag="xch")
                        nc.sync.dma_start(
                            out=xt[:],
                            in_=(xT_own if own else xT)[:, ch * P:(ch + 1) * P])
                        src_ap = xt[:]
                    else:
                        if own:
                            src_ap = hT_own[:, ch * P:(ch + 1) * P]
                        else:
                            slab = wp.tile([H, P], BF, tag="hslab")
                            nc.sync.dma_start(
                                out=slab[:],
                                in_=hT_agD[ch // NCH_OWN, :,
                                           (ch % NCH_OWN) * P:(ch % NCH_OWN + 1) * P])
                            src_ap = slab[:]
                    nc.vector.tensor_scalar(
                        out=lhs[:KD, :], in0=src_ap,
                        scalar1=stats[:KD, 4:5], scalar2=stats[:KD, 5:6],
                        op0=ALU.mult, op1=ALU.add)
                    ps = psp.tile([P, 192], FP, tag="tb")
                    if li == 0:
                        r0, r1, r2 = w0[0][:], w0[1][:], w0[2][:]
                    else:
                        r0, r1, r2 = (wly[li - 1][0][:], wly[li - 1][1][:],
                                      wly[li - 1][2][:])
                    nc.tensor.matmul(out=ps[:, 0:64], lhsT=lhs[:KD + 1, :],
                                     rhs=r0[:KD + 1, :], start=True, stop=True)
                    nc.tensor.matmul(out=ps[:, 64:128], lhsT=lhs[:KD + 1, :],
                                     rhs=r1[:KD + 1, :], start=True, stop=True)
                    nc.tensor.matmul(out=ps[:, 128:192], lhsT=lhs[:KD + 1, :],
                                     rhs=r2[:KD + 1, :], start=True, stop=True)
                    base = ch * P if own else n_pad + ch * P
                    if li == 0:
                        u1t = wp.tile([P, H], FP, tag="u1t")
                        nc.vector.tensor_copy(u1t[:], ps[:, 64:128])
                        nc.sync.dma_start(out=U1[base:base + P, :], in_=u1t[:])
                        if own:
                            nc.vector.tensor_copy(B_sb[:, ch, :], ps[:, 128:192])
                        h1t = wp.tile([P, H], FP, tag="h1t")
                        nc.scalar.activation(out=h1t[:], in_=ps[:, 0:64],
                                             func=ACTF.Relu)
                        nc.vector.tensor_scalar(
                            out=h1t[:], in0=h1t[:],
                            scalar1=(masko if own else maska)[:, ch:ch + 1],
                            scalar2=None, op0=ALU.mult)
                        tp = pst.tile([H, P], FP, tag="tpx")
                        nc.tensor.transpose(out=tp[:], in_=h1t[:], identity=idt[:])
                        if own:
                            nc.vector.tensor_copy(
                                hT_own[:, ch * P:(ch + 1) * P], tp[:])
                        else:
                            hbf = wp.tile([H, P], BF, tag="hbf")
                            nc.vector.tensor_copy(hbf[:], tp[:])
                            nc.sync.dma_start(
                                out=hT_agD[ch // NCH_OWN, :,
                                           (ch % NCH_OWN) * P:(ch % NCH_OWN + 1) * P],
                                in_=hbf[:])
                    else:
                        t1t = wp.tile([P, 2 * H], FP, tag="t1t")
                        nc.vector.tensor_scalar(
                            out=t1t[:, 0:64], in0=ps[:, 0:64],
                            scalar1=(dinvo if own else dinva)[:, ch:ch + 1],
                            scalar2=None, op0=ALU.mult)
                        nc.vector.tensor_copy(t1t[:, 64:128], ps[:, 64:128])
                        nc.sync.dma_start(out=T1[base:base + P, :], in_=t1t[:])
                        if own:
                            nc.vector.tensor_copy(B_sb[:, ch, :], ps[:, 128:192])

            # ================= layer 0 =================
            # x stats
            CW = 2048
            nxc = (NT + CW - 1) // CW
            for ci_ in range(nxc):
                w = min(CW, NT - ci_ * CW)
                xc = wp.tile([F_NODE, CW], FP, tag="xstat")
                nc.sync.dma_start(out=xc[:, :w], in_=xT[:, ci_ * CW:ci_ * CW + w])
                t1 = wp.tile([H, 1], FP, tag="st1")
                nc.vector.tensor_reduce(out=t1[:F_NODE, :], in_=xc[:, :w],
                                        axis=AX.X, op=ALU.add)
                nc.vector.tensor_tensor(out=xc[:, :w], in0=xc[:, :w],
                                        in1=xc[:, :w], op=ALU.mult)
                t2 = wp.tile([H, 1], FP, tag="st2")
                nc.vector.tensor_reduce(out=t2[:F_NODE, :], in_=xc[:, :w],
                                        axis=AX.X, op=ALU.add)
                if ci_ == 0:
                    nc.vector.tensor_copy(stats[:F_NODE, 0:1], t1[:F_NODE, :])
                    nc.vector.tensor_copy(stats[:F_NODE, 1:2], t2[:F_NODE, :])
                else:
                    nc.vector.tensor_tensor(out=stats[:F_NODE, 0:1],
                                            in0=stats[:F_NODE, 0:1],
                                            in1=t1[:F_NODE, :], op=ALU.add)
                    nc.vector.tensor_tensor(out=stats[:F_NODE, 1:2],
                                            in0=stats[:F_NODE, 1:2],
                                            in1=t2[:F_NODE, :], op=ALU.add)
            bn_from_stats(F_NODE, bnx[:], float(N_NODES))
            table_pass(0, True)
            table_pass(0, False)
            nc.sync.dma_start(out=U1[ZR:ZR + 1, :], in_=zrow[0:1, 0:H])

            # layer-0 edge phase: build er and Ce_i (per 8-col block)
            for (D, nb, slot0, cc0) in gmeta:
                gidx = slot0 // P
                for b_ in range(nb):
                    nreal = min(7, D - b_ * 7)
                    gtb = gp.tile([P, 8, H], FP, tag="gt0")
                    o = (cc0 + b_ * 8) * 8
                    nc.gpsimd.dma_gather(
                        out_ap=gtb[:], in_ap=U1[BIAS16:], idxs_ap=it[:, o:o + 64],
                        num_idxs=1024, num_idxs_reg=1024, elem_size=H,
                        queue_num=b_ % 2)
                    eab = wp.tile([F_EDGE, 8 * P], FP, tag="ea")
                    nc.sync.dma_start(
                        out=eab[:],
                        in_=eaT[:, (cc0 + b_ * 8) * P:(cc0 + b_ * 8 + 8) * P])
                    etb = wp.tile([P, 8, H], FP, tag="et")
                    for jj in range(nreal):
                        pse = pst.tile([P, H], FP, tag="tpx")
                        nc.tensor.matmul(out=pse[:],
                                         lhsT=eab[:, jj * P:(jj + 1) * P],
                                         rhs=wce0[:], start=True, stop=True)
                        nc.vector.tensor_tensor(out=etb[:, jj, :],
                                                in0=gtb[:, jj, :], in1=pse[:],
                                                op=ALU.add)
                    nc.vector.tensor_tensor(
                        out=etb[:, 0:nreal, :], in0=etb[:, 0:nreal, :],
                        in1=B_sb[:, gidx:gidx + 1, :].to_broadcast([P, nreal, H]),
                        op=ALU.add)
                    nc.scalar.activation(out=etb[:, 0:nreal, :],
                                         in_=etb[:, 0:nreal, :], func=ACTF.Relu)
                    ce3b = wp.tile([P, 8, 192], FP, tag="ce3")
                    for jj in range(nreal):
                        tp = pst.tile([H, P], FP, tag="tpx")
                        nc.tensor.transpose(out=tp[:], in_=etb[:, jj, :],
                                            identity=idt[:])
                        erT = wp.tile([H, P], FP, tag="erT")
                        nc.vector.tensor_copy(erT[:], tp[:])
                        psc = psp.tile([P, 192], FP, tag="tb")
                        for i3 in range(3):
                            nc.tensor.matmul(out=psc[:, i3 * 64:(i3 + 1) * 64],
                                             lhsT=erT[:], rhs=wce[i3][:],
                                             start=True, stop=True)
                        nc.vector.tensor_copy(ce3b[:, jj, :], psc[:])
                    for i3 in range(3):
                        nc.sync.dma_start(
                            out=CeD[i3, :, cc0 + b_ * 8:cc0 + b_ * 8 + 8, :],
                            in_=ce3b[:, :, i3 * 64:(i3 + 1) * 64])

            # h1 stats (hT_agD was filled during full pass)
            stats_from_hT_ag()

            # ================= layers 1..3 =================
            gps = psg.tile([H, H], FP, name="gps", tag="hd", padded_shape=[H, 512])   # pooling accum
            for li in range(1, 4):
                bn_from_stats(H, bnly[:, (li - 1) * 2:(li - 1) * 2 + 2],
                              float(N_NODES))
                table_pass(li, True)
                table_pass(li, False)
                nc.sync.dma_start(out=T1[ZR:ZR + 1, :], in_=zrow[0:1, :])

                for gi_, (D, nb, slot0, cc0) in enumerate(gmeta):
                    gidx = slot0 // P
                    hacc = wp.tile([P, H], FP, tag="hacc")
                    for b_ in range(nb):
                        nreal = min(7, D - b_ * 7)
                        gtb = gp.tile([P, 8, 2 * H], FP, tag="gt")
                        o = (cc0 + b_ * 8) * 8
                        nc.gpsimd.dma_gather(
                            out_ap=gtb[:], in_ap=T1[BIAS16:],
                            idxs_ap=it[:, o:o + 64],
                            num_idxs=1024, num_idxs_reg=1024, elem_size=2 * H,
                            queue_num=b_ % 2)
                        ceb = gp.tile([P, 8, H], FP, tag="cet")
                        nc.sync.dma_start(
                            out=ceb[:],
                            in_=CeD[li - 1, :, cc0 + b_ * 8:cc0 + b_ * 8 + 8, :])
                        nc.vector.tensor_tensor(
                            out=ceb[:, 0:nreal, :], in0=ceb[:, 0:nreal, :],
                            in1=gtb[:, 0:nreal, H:2 * H], op=ALU.add)
                        nc.vector.tensor_tensor(
                            out=ceb[:, 0:nreal, :], in0=ceb[:, 0:nreal, :],
                            in1=B_sb[:, gidx:gidx + 1, :].to_broadcast(
                                [P, nreal, H]),
                            op=ALU.add)
                        nc.scalar.activation(out=ceb[:, 0:nreal, :],
                                             in_=ceb[:, 0:nreal, :],
                                             func=ACTF.Sigmoid)
                        nc.vector.tensor_tensor(
                            out=ceb[:, 0:nreal, :], in0=ceb[:, 0:nreal, :],
                            in1=gtb[:, 0:nreal, 0:H], op=ALU.mult)
                        rr = wp.tile([P, H], FP, tag="rr")
                        nc.vector.tensor_reduce(
                            out=rr[:],
                            in_=ceb[:, 0:nreal, :].transpose([0, 2, 1]),
                            axis=AX.X, op=ALU.add)
                        if b_ == 0:
                            nc.vector.tensor_copy(hacc[:], rr[:])
                        else:
                            nc.vector.tensor_tensor(out=hacc[:], in0=hacc[:],
                                                    in1=rr[:], op=ALU.add)
                    nc.scalar.activation(out=hacc[:], in_=hacc[:], func=ACTF.Relu)
                    nc.vector.tensor_scalar(out=hacc[:], in0=hacc[:],
                                            scalar1=dinvo[:, gidx:gidx + 1],
                                            scalar2=None, op0=ALU.mult)
                    if li == 3:
                        gpm = wp.tile([P, H], FP, tag="gpm")
                        nc.vector.tensor_scalar(out=gpm[:], in0=iota64f[:],
                                                scalar1=batv[:, gidx:gidx + 1],
                                                scalar2=None, op0=ALU.is_equal)
                        nc.tensor.matmul(out=gps[:], lhsT=hacc[:], rhs=gpm[:],
                                         start=(gi_ == 0),
                                         stop=(gi_ == len(gmeta) - 1),
                                         skip_group_check=True)
                    else:
                        tp = pst.tile([H, P], FP, tag="tpx")
                        nc.tensor.transpose(out=tp[:], in_=hacc[:],
                                            identity=idt[:])
                        nc.vector.tensor_copy(hT_own[:, slot0:slot0 + P], tp[:])

                if li < 3:
                    hbf = wp.tile([H, n_pad], BF, tag="agbf")
                    nc.vector.tensor_copy(hbf[:], hT_own[:])
                    nc.sync.dma_start(out=ag_in[:], in_=hbf[:])
                    nc.gpsimd.collective_compute(
                        "AllGather", ALU.bypass, replica_groups=RG,
                        ins=[ag_in[:]], outs=[ag_out[:]])
                    for r in range(N_CORES):
                        nc.sync.dma_start(out=hT_agD[r],
                                          in_=ag_out[r * H:(r + 1) * H, :])
                    stats_from_hT_ag()

            # ================= head =================
            gsb = wp.tile([H, H], FP, tag="gsb")
            nc.vector.tensor_copy(gsb[:], gps[:])
            nc.sync.dma_start(out=g_in[:], in_=gsb[:, 0:GPC])
            nc.gpsimd.collective_compute(
                "AllGather", ALU.bypass, replica_groups=RG,
                ins=[g_in[:]], outs=[g_out[:]])
            gT = pp.tile([H + 1, N_GRAPHS], FP)
            nc.vector.memset(gT[H:H + 1, :], 1.0)
            for r in range(N_CORES):
                nc.sync.dma_start(out=gT[:H, r * GPC:(r + 1) * GPC],
                                  in_=g_out[r * H:(r + 1) * H, :])
            # BN over 512 graphs
            nc.vector.tensor_reduce(out=stats[:, 0:1], in_=gT[:H, :], axis=AX.X,
                                    op=ALU.add)
            sq5 = wp.tile([H, N_GRAPHS], FP, tag="sq5")
            nc.scalar.activation(out=sq5[:], in_=gT[:H, :], func=ACTF.Square)
            nc.vector.tensor_reduce(out=stats[:, 1:2], in_=sq5[:], axis=AX.X,
                                    op=ALU.add)
            bn_from_stats(H, bnhd[:, 0:2], float(N_GRAPHS))
            gbn = pp.tile([H + 1, N_GRAPHS], FP)
            nc.vector.memset(gbn[H:H + 1, :], 1.0)
            nc.vector.tensor_scalar(out=gbn[:H, :], in0=gT[:H, :],
                                    scalar1=stats[:, 4:5], scalar2=stats[:, 5:6],
                                    op0=ALU.mult, op1=ALU.add)
            ph = psg.tile([H, N_GRAPHS], FP, name="ph", tag="hd")
            nc.tensor.matmul(out=ph[:], lhsT=whd[0][:], rhs=gbn[:], start=True,
                             stop=True)
            nc.scalar.activation(out=gT[:H, :], in_=ph[:], func=ACTF.Relu)
            nc.vector.tensor_reduce(out=stats[:, 0:1], in_=gT[:H, :], axis=AX.X,
                                    op=ALU.add)
            nc.scalar.activation(out=sq5[:], in_=gT[:H, :], func=ACTF.Square)
            nc.vector.tensor_reduce(out=stats[:, 1:2], in_=sq5[:], axis=AX.X,
                                    op=ALU.add)
            bn_from_stats(H, bnhd[:, 2:4], float(N_GRAPHS))
            nc.vector.tensor_scalar(out=gbn[:H, :], in0=gT[:H, :],
                                    scalar1=stats[:, 4:5], scalar2=stats[:, 5:6],
                                    op0=ALU.mult, op1=ALU.add)
            pl = psg.tile([H, N_GRAPHS], FP, name="pl", tag="hd")
            nc.tensor.matmul(out=pl[:], lhsT=whd[1][:], rhs=gbn[:], start=True,
                             stop=True)
            lgT = pp.tile([H, N_GRAPHS], FP)
            nc.vector.tensor_copy(lgT[:], pl[:])
            # log_softmax per graph: transpose chunks of 128 graphs
            for cg in range(N_GRAPHS // P):
                tp = pst.tile([P, P], FP, tag="tpx")
                nc.tensor.transpose(out=tp[:, 0:N_CLASSES],
                                    in_=lgT[0:N_CLASSES, cg * P:(cg + 1) * P],
                                    identity=idt[0:N_CLASSES, 0:N_CLASSES])
                lg = wp.tile([P, N_CLASSES], FP, tag="lg")
                nc.vector.tensor_copy(lg[:], tp[:, 0:N_CLASSES])
                mx = wp.tile([P, 1], FP, tag="mx")
                nc.vector.tensor_reduce(out=mx[:], in_=lg[:], axis=AX.X,
                                        op=ALU.max)
                nc.vector.tensor_scalar(out=lg[:], in0=lg[:], scalar1=mx[:],
                                        scalar2=None, op0=ALU.subtract)
                ex = wp.tile([P, N_CLASSES], FP, tag="ex")
                sm = wp.tile([P, 1], FP, tag="sm")
                nc.scalar.activation(out=ex[:], in_=lg[:], func=ACTF.Exp,
                                     accum_out=sm[:])
                nc.scalar.activation(out=sm[:], in_=sm[:], func=ACTF.Ln)
                nc.vector.tensor_scalar(out=lg[:], in0=lg[:], scalar1=sm[:],
                                        scalar2=None, op0=ALU.subtract)
                nc.sync.dma_start(out=out[cg * P:(cg + 1) * P, :], in_=lg[:])

    nc.finalize()
    return nc


# --------------------------------------------------------------- run + host --
_CACHE = {}
_RUNNERS = {}


def _get_runner(nc, key):
    """jit-once SPMD runner (no donation, reusable across calls)."""
    if key in _RUNNERS:
        return _RUNNERS[key]
    import jax
    from jax.sharding import Mesh, PartitionSpec
    from jax.experimental.shard_map import shard_map
    from concourse import bass2jax
    bass2jax.install_neuronx_cc_hook()
    partition_name = nc.partition_id_tensor.name if nc.partition_id_tensor else None
    in_names, out_names, out_avals, zero_outs = [], [], [], []
    for alloc in nc.m.functions[0].allocations:
        if not isinstance(alloc, mybir.MemoryLocationSet):
            continue
        name = alloc.memorylocations[0].name
        if alloc.kind == "ExternalInput":
            if name != partition_name:
                in_names.append(name)
        elif alloc.kind == "ExternalOutput":
            out_names.append(name)
            shape = tuple(alloc.tensor_shape)
            dtype = mybir.dt.np(alloc.dtype)
            out_avals.append(jax.core.ShapedArray(shape, dtype))
            zero_outs.append(np.zeros((N_CORES * shape[0],) + shape[1:], dtype))
    all_in = in_names + out_names
    if partition_name is not None:
        all_in = all_in + [partition_name]

    def _body(*args):
        operands = list(args)
        if partition_name is not None:
            operands.append(bass2jax.partition_id_tensor())
        outs = bass2jax._bass_exec_p.bind(
            *operands, out_avals=tuple(out_avals), in_names=tuple(all_in),
            out_names=tuple(out_names), lowering_input_output_aliases=(),
            sim_require_finite=True, sim_require_nnan=True, nc=nc)
        return tuple(outs)

    devices = jax.devices()[:N_CORES]
    mesh = Mesh(np.asarray(devices), ("core",))
    n_ops = len(in_names) + len(zero_outs)
    sharded = jax.jit(
        shard_map(_body, mesh=mesh,
                  in_specs=(PartitionSpec("core"),) * n_ops,
                  out_specs=(PartitionSpec("core"),) * len(out_names),
                  check_rep=False),
        keep_unused=True)

    def run(concat_in):
        return sharded(*[concat_in[n] for n in in_names], *zero_outs)

    _RUNNERS[key] = (run, in_names, out_names)
    return _RUNNERS[key]


def _build_in_maps(inp, meta):
    n_pad, NT, ZR = meta["n_pad"], meta["NT"], meta["ZR"]
    NCH_OWN, NCH_ALL = n_pad // P, NT // P
    n_cc = meta["n_cc"]

    x_pad = np.zeros((NT, F_NODE), np.float32)
    deg_pad = np.zeros(NT, np.float32)
    mask_pad = np.zeros(NT, np.float32)
    for c, ci in enumerate(meta["cores"]):
        b0 = c * n_pad
        sel = ci["x_rows"] >= 0
        ii = np.nonzero(sel)[0]
        x_pad[b0 + ii] = inp["x"][ci["x_rows"][ii]]
        deg_pad[b0:b0 + n_pad] = ci["deg_slot"]
        mask_pad[b0 + ii] = 1.0
    dinv_pad = (1.0 / np.sqrt(np.maximum(deg_pad, 1.0))).astype(np.float32)

    def cols(a, nch):
        return np.ascontiguousarray(a.reshape(nch, P).T).astype(np.float32)

    xT = np.ascontiguousarray(x_pad.T).astype(np.float32)
    dinv_all = cols(dinv_pad, NCH_ALL)
    mask_all = cols(mask_pad, NCH_ALL)

    Wn0, bn0 = inp["Wn0"], inp["bn0"]
    We0 = inp["We0"]
    w_l0 = np.stack([
        np.vstack([Wn0, bn0[None, :]]),
        np.vstack([Wn0 @ We0[:H], (bn0 @ We0[:H])[None, :]]),
        np.vstack([Wn0 @ We0[H:2 * H],
                   (bn0 @ We0[H:2 * H] + inp["be0"])[None, :]]),
    ]).astype(np.float32)
    w_ly = np.zeros((3, 3, H + 1, H), np.float32)
    for i in range(3):
        Wn, Wnb, We, Web = inp["Wn"][i], inp["Wnb"][i], inp["We"][i], inp["Web"][i]
        w_ly[i, 0] = np.vstack([Wn, Wnb[None, :]])
        w_ly[i, 1] = np.vstack([Wn @ We[:H], (Wnb @ We[:H])[None, :]])
        w_ly[i, 2] = np.vstack([Wn @ We[H:2 * H],
                                (Wnb @ We[H:2 * H] + Web)[None, :]])
    w_ce0 = We0[2 * H:].astype(np.float32)
    w_ce = np.stack([inp["We"][i, 2 * H:] for i in range(3)]).astype(np.float32)
    w_hd = np.zeros((2, H + 1, H), np.float32)
    w_hd[0] = np.vstack([inp["Wfc"], inp["bfc"][None, :]])
    w_hd[1, :H, :N_CLASSES] = inp["Wcls"]
    w_hd[1, H, :N_CLASSES] = inp["bcls"]
    bn_ly = np.zeros((H, 8), np.float32)
    for i in range(3):
        bn_ly[:, 2 * i] = inp["bns_w"][i]
        bn_ly[:, 2 * i + 1] = inp["bns_b"][i]
    bn_x = np.stack([inp["bn_feat_w"], inp["bn_feat_b"]], 1).astype(np.float32)
    bn_hd = np.stack([inp["bn_fc_w"], inp["bn_fc_b"],
                      inp["bn_hid_w"], inp["bn_hid_b"]], 1).astype(np.float32)

    in_maps = []
    for c, ci in enumerate(meta["cores"]):
        b0 = c * n_pad
        x_own_pad = x_pad[b0:b0 + n_pad]
        ea_s = np.zeros((n_cc * P, F_EDGE), np.float32)
        se = ci["stream_eid"]
        v = se >= 0
        ea_s[v] = inp["edge_attr"][se[v]]
        in_maps.append(dict(
            xT_own=np.ascontiguousarray(x_own_pad.T).astype(np.float32),
            xT=xT,
            idxs=wrap_idx16(ci["tidx"]),
            eaT=np.ascontiguousarray(ea_s.T).astype(np.float32),
            dinv_own=cols(dinv_pad[b0:b0 + n_pad], NCH_OWN),
            dinv_all=dinv_all,
            mask_own=cols(mask_pad[b0:b0 + n_pad], NCH_OWN),
            mask_all=mask_all,
            bat=cols(ci["bat_slot"], NCH_OWN),
            w_l0=w_l0, w_ly=w_ly, w_ce0=w_ce0, w_ce=w_ce, w_hd=w_hd,
            bn_ly=bn_ly, bn_x=bn_x, bn_hd=bn_hd,
        ))
    return in_maps


def kernel(**inputs):
    inp = {k: np.asarray(v) for k, v in inputs.items()}
    meta = prep(inp["edge_index"], inp["batch"])
    key = (meta["n_pad"], meta["NT"], meta["n_cc"], tuple(meta["tmpl"]))
    if key not in _CACHE:
        _CACHE[key] = build_nc(meta)
    nc = _CACHE[key]
    in_maps = _build_in_maps(inp, meta)
    run, in_names, out_names = _get_runner(nc, key)
    concat_in = {n: np.concatenate([np.asarray(m[n]) for m in in_maps], axis=0)
                 for n in in_names}
    outs = run(concat_in)
    oi = out_names.index("out")
    return np.asarray(outs[oi][:N_GRAPHS], np.float32)


def prepare_timed(**inputs):
    """Build everything once; return a zero-host-cost repeat runner."""
    import jax
    out = kernel(**inputs)  # ensures cache + jit built
    inp = {k: np.asarray(v) for k, v in inputs.items()}
    meta = prep(inp["edge_index"], inp["batch"])
    key = (meta["n_pad"], meta["NT"], meta["n_cc"], tuple(meta["tmpl"]))
    nc = _CACHE[key]
    run, in_names, out_names = _get_runner(nc, key)
    in_maps = _build_in_maps(inp, meta)
    concat_in = {n: jax.device_put(
        np.concatenate([np.asarray(m[n]) for m in in_maps], axis=0))
        for n in in_names}

    def timed_run():
        outs = run(concat_in)
        jax.block_until_ready(outs)
        return outs

    return timed_run, out


# revision 14
# speedup vs baseline: 406.7302x; 3.7857x over previous
"""EGCNNet Trainium2 kernel: 8-core SPMD Bass implementation (self-contained)."""
import numpy as np

from concourse import bacc, bass, mybir, tile
from concourse.bass_utils import run_bass_kernel_spmd

N_NODES = 50000
N_EDGES = 800000
N_GRAPHS = 512
H = 64
F_NODE = 32
F_EDGE = 8
N_CLASSES = 10
EPS = 1e-5
N_CORES = 8
P = 128
BIAS16 = 32768
FP = mybir.dt.float32
BF = mybir.dt.bfloat16
AX = mybir.AxisListType
ALU = mybir.AluOpType
ACTF = mybir.ActivationFunctionType
RG = [list(range(N_CORES))]


# ---------------------------------------------------------------- host prep --
def _choose_buckets(hists):
    dmax = hists.shape[1] - 1
    cum = np.cumsum(hists[:, 1:], axis=1)

    def nodes(c, a, b):
        return cum[c, b - 1] - (cum[c, a - 2] if a >= 2 else 0)

    best = np.full(dmax + 1, np.inf)
    best[0] = 0.0
    choice = np.zeros(dmax + 1, np.int64)
    for b in range(1, dmax + 1):
        for a in range(1, b + 1):
            g = max(int(np.ceil(nodes(c, a, b) / P)) for c in range(N_CORES))
            cost = best[a - 1] + g * P * (int(np.ceil(b / 7.0)) * 8)
            if cost < best[b]:
                best[b] = cost
                choice[b] = a
    buckets = []
    b = dmax
    while b >= 1:
        a = int(choice[b])
        buckets.append((a, b))
        b = a - 1
    return buckets[::-1]


def prep(edge_index, batch):
    src = np.asarray(edge_index[0], np.int64)
    dst = np.asarray(edge_index[1], np.int64)
    batch = np.asarray(batch, np.int64)
    gpc = N_GRAPHS // N_CORES
    node_start = np.searchsorted(batch, np.arange(0, N_GRAPHS + 1, gpc))
    deg_global = np.bincount(dst, minlength=N_NODES).astype(np.int64)
    dmax = int(deg_global.max())
    hists = np.zeros((N_CORES, dmax + 1), np.int64)
    for c in range(N_CORES):
        lo, hi = int(node_start[c]), int(node_start[c + 1])
        hists[c] = np.bincount(deg_global[lo:hi], minlength=dmax + 1)
    buckets = _choose_buckets(hists)

    tmpl = []
    for (a, b) in buckets:
        g = max(int(np.ceil(hists[c, a:b + 1].sum() / P)) for c in range(N_CORES))
        if g > 0:
            tmpl.append((a, b, g))
    n_groups = sum(g for _, _, g in tmpl)
    nzero_max = max(int(hists[c, 0]) for c in range(N_CORES))
    zero_groups = int(np.ceil((nzero_max + 1) / P))
    n_pad = (n_groups + zero_groups) * P
    NT = N_CORES * n_pad
    TROWS = n_pad + NT
    assert TROWS <= 65536, f"table rows {TROWS} > 65536"
    ZR = n_groups * P

    # groups: (D, nb, slot0, c0) ; c0 in padded chunkcol units (8 per block)
    gmeta = []
    gbase = 0
    c0 = 0
    for (a, b, G) in tmpl:
        nb = int(np.ceil(b / 7.0))
        for g in range(G):
            gmeta.append((b, nb, (gbase + g) * P, c0))
            c0 += nb * 8
        gbase += G
    n_cc = c0  # padded chunkcols

    cores = []
    for c in range(N_CORES):
        lo, hi = int(node_start[c]), int(node_start[c + 1])
        nk = hi - lo
        deg = deg_global[lo:hi]
        em = (dst >= lo) & (dst < hi)
        e_ids = np.nonzero(em)[0]
        dl = dst[e_ids] - lo
        eo = np.lexsort((src[e_ids], dl))
        e_ids = e_ids[eo]
        dl = dl[eo]
        run_start = np.zeros(nk + 1, np.int64)
        np.cumsum(np.bincount(dl, minlength=nk), out=run_start[1:])

        old2slot = np.full(nk, -1, np.int64)
        gi = 0
        for (a, b, G) in tmpl:
            ids = np.nonzero((deg >= a) & (deg <= b))[0]
            for g in range(G):
                chunk = ids[g * P:(g + 1) * P]
                slot0 = gmeta[gi][2]
                old2slot[chunk] = slot0 + np.arange(len(chunk))
                gi += 1
        ids0 = np.nonzero(deg == 0)[0]
        old2slot[ids0] = ZR + 1 + np.arange(len(ids0))
        assert (old2slot >= 0).all()

        stream_eid = np.full(n_cc * P, -2, np.int64)  # -2 sentinel, -1 ZR-dummy
        slot_of = np.full(n_pad, -1, np.int64)
        slot_of[old2slot] = np.arange(nk)
        for (D, nb, slot0, cc0) in gmeta:
            nid = slot_of[slot0:slot0 + P]
            ok = nid >= 0
            d_n = np.where(ok, deg[np.maximum(nid, 0)], 0)
            for blk in range(nb):
                for jj in range(7):
                    j = blk * 7 + jj
                    cc = cc0 + blk * 8 + jj
                    if j >= D:
                        stream_eid[cc * P:(cc + 1) * P] = -1
                        continue
                    lane_ok = ok & (j < d_n)
                    e_sel = run_start[np.maximum(nid, 0)] + j
                    vals = np.where(lane_ok,
                                    e_ids[np.minimum(e_sel, max(len(e_ids) - 1, 0))],
                                    -1)
                    stream_eid[cc * P:(cc + 1) * P] = vals
                # 8th col stays -2 (sentinel)
        cores.append(dict(lo=lo, hi=hi, nk=nk, old2slot=old2slot,
                          stream_eid=stream_eid, deg=deg))

    old2new = np.zeros(N_NODES, np.int64)
    for c, ci in enumerate(cores):
        old2new[np.arange(ci["lo"], ci["hi"])] = c * n_pad + ci["old2slot"]

    for c, ci in enumerate(cores):
        se = ci["stream_eid"]
        gsrc = old2new[src[np.maximum(se, 0)]]
        own = gsrc // n_pad == c
        tpos = np.where(own, gsrc % n_pad, n_pad + gsrc)
        tpos = np.where(se >= 0, tpos, ZR)          # ZR-dummies & sentinel -> ZR
        tpos = np.where(se == -2, TROWS - 1, tpos)  # sentinel -> high row
        ci["tidx"] = tpos
        deg_slot = np.zeros(n_pad, np.float32)
        bat_slot = np.zeros(n_pad, np.float32)
        x_rows = np.full(n_pad, -1, np.int64)
        deg_slot[ci["old2slot"]] = ci["deg"]
        bat_slot[ci["old2slot"]] = (batch[ci["lo"]:ci["hi"]] - c * gpc)
        x_rows[ci["old2slot"]] = np.arange(ci["lo"], ci["hi"])
        ci["deg_slot"] = deg_slot
        ci["bat_slot"] = bat_slot
        ci["x_rows"] = x_rows

    return dict(cores=cores, tmpl=tmpl, gmeta=gmeta, n_pad=n_pad, NT=NT, ZR=ZR,
                n_cc=n_cc, node_start=node_start, TROWS=TROWS)


def wrap_idx16(idx):
    k = idx.shape[0]
    b = (idx - BIAS16).astype(np.int16)
    return np.tile(b.reshape(k // 16, 16).T, (8, 1)).copy()


# ------------------------------------------------------------ bass builder --
def build_nc(meta):
    import os
    PH = os.environ.get("K_PHASES", "all")
    n_pad, NT, ZR, n_cc, TROWS = (meta["n_pad"], meta["NT"], meta["ZR"],
                                  meta["n_cc"], meta["TROWS"])
    gmeta = meta["gmeta"]
    NCH_OWN = n_pad // P
    NCH_ALL = NT // P
    GPC = N_GRAPHS // N_CORES

    nc = bacc.Bacc("TRN2", target_bir_lowering=False, debug=False,
                   num_swdge_queues=2)

    def din(name, shape, dt=FP):
        return nc.declare_dram_parameter(name, list(shape), dt, isOutput=False)

    xT_own = din("xT_own", [F_NODE, n_pad])
    xT = din("xT", [F_NODE, NT])
    idxs = din("idxs", [P, n_cc * 8], mybir.dt.int16)
    eaT = din("eaT", [F_EDGE, n_cc * P])
    dinv_own = din("dinv_own", [P, NCH_OWN])
    dinv_all = din("dinv_all", [P, NCH_ALL])
    mask_own = din("mask_own", [P, NCH_OWN])
    mask_all = din("mask_all", [P, NCH_ALL])
    bat = din("bat", [P, NCH_OWN])
    w_l0 = din("w_l0", [3, F_NODE + 1, H])
    w_ly = din("w_ly", [3, 3, H + 1, H])
    w_ce0 = din("w_ce0", [F_EDGE, H])
    w_ce = din("w_ce", [3, H, H])
    w_hd = din("w_hd", [2, H + 1, H])
    bn_ly = din("bn_ly", [H, 8])   # cols: (w,b) x layers 1..3, pad
    bn_x = din("bn_x", [F_NODE, 2])
    bn_hd = din("bn_hd", [H, 4])
    out = nc.declare_dram_parameter("out", [N_GRAPHS, N_CLASSES], FP,
                                    isOutput=True)

    T1 = nc.dram_tensor("T1", [TROWS, 2 * H], FP)
    U1 = nc.dram_tensor("U1", [TROWS, H], FP)
    CeD = nc.dram_tensor("CeD", [3, P, n_cc, H], FP)
    ag_in = nc.dram_tensor("ag_in", [H, n_pad], BF)
    ag_out = nc.dram_tensor("ag_out", [N_CORES * H, n_pad], BF,
                            addr_space="Shared")
    hT_agD = nc.dram_tensor("hT_agD", [N_CORES, H, n_pad], BF)
    g_in = nc.dram_tensor("g_in", [H, GPC], FP)
    g_out = nc.dram_tensor("g_out", [N_CORES * H, GPC], FP, addr_space="Shared")

    with tile.TileContext(nc) as tc:
        with tc.tile_pool(name="persist", bufs=1) as pp, \
             tc.tile_pool(name="work", bufs=2) as wp, \
             tc.tile_pool(name="gath", bufs=2) as gp, \
             tc.tile_pool(name="ps", bufs=2, space="PSUM") as psp, \
             tc.tile_pool(name="pst", bufs=2, space="PSUM") as pst, \
             tc.tile_pool(name="psg", bufs=1, space="PSUM") as psg:

            it = pp.tile([P, n_cc * 8], mybir.dt.int16)
            nc.sync.dma_start(out=it[:], in_=idxs[:])
            hT_own = pp.tile([H, n_pad], FP)
            B_sb = pp.tile([P, NCH_OWN, H], FP)
            dinvo = pp.tile([P, NCH_OWN], FP)
            nc.sync.dma_start(out=dinvo[:], in_=dinv_own[:])
            dinva = pp.tile([P, NCH_ALL], FP)
            nc.sync.dma_start(out=dinva[:], in_=dinv_all[:])
            masko = pp.tile([P, NCH_OWN], FP)
            nc.sync.dma_start(out=masko[:], in_=mask_own[:])
            maska = pp.tile([P, NCH_ALL], FP)
            nc.sync.dma_start(out=maska[:], in_=mask_all[:])
            batv = pp.tile([P, NCH_OWN], FP)
            nc.sync.dma_start(out=batv[:], in_=bat[:])
            iota64 = pp.tile([P, H], mybir.dt.int32)
            nc.gpsimd.iota(iota64[:], pattern=[[1, H]], base=0,
                           channel_multiplier=0)
            iota64f = pp.tile([P, H], FP)
            nc.vector.tensor_copy(iota64f[:], iota64[:])
            idt = pp.tile([P, P], FP)
            from concourse.masks import make_identity
            make_identity(nc, idt[:])
            w0 = [pp.tile([F_NODE + 1, H], FP, name=f"w0_{k}", tag=f"w0_{k}") for k in range(3)]
            for k in range(3):
                nc.sync.dma_start(out=w0[k][:], in_=w_l0[k])
            wly = [[pp.tile([H + 1, H], FP, name=f"wly_{i}_{k}", tag=f"wly_{i}_{k}") for k in range(3)]
                   for i in range(3)]
            for i in range(3):
                for k in range(3):
                    nc.sync.dma_start(out=wly[i][k][:], in_=w_ly[i, k])
            wce0 = pp.tile([F_EDGE, H], FP)
            nc.sync.dma_start(out=wce0[:], in_=w_ce0[:])
            wce = [pp.tile([H, H], FP, name=f"wce_{k}", tag=f"wce_{k}") for k in range(3)]
            for k in range(3):
                nc.sync.dma_start(out=wce[k][:], in_=w_ce[k])
            whd = [pp.tile([H + 1, H], FP, name=f"whd_{k}", tag=f"whd_{k}") for k in range(2)]
            for k in range(2):
                nc.sync.dma_start(out=whd[k][:], in_=w_hd[k])
            bnly = pp.tile([H, 8], FP)
            nc.sync.dma_start(out=bnly[:], in_=bn_ly[:])
            bnx = pp.tile([F_NODE, 2], FP)
            nc.sync.dma_start(out=bnx[:], in_=bn_x[:])
            bnhd = pp.tile([H, 4], FP)
            nc.sync.dma_start(out=bnhd[:], in_=bn_hd[:])
            zrow = pp.tile([1, 2 * H], FP)
            nc.vector.memset(zrow[:], 0.0)
            epst = pp.tile([H, 1], FP)
            nc.vector.memset(epst[:], EPS)
            stats = pp.tile([H, 8], FP)
            lhs_a = pp.tile([H + 1, 4 * P], FP)
            lhs_b = pp.tile([H + 1, 4 * P], FP)
            nc.vector.memset(lhs_a[H:H + 1, :], 1.0)
            nc.vector.memset(lhs_b[H:H + 1, :], 1.0)
            nc.vector.memset(lhs_a[F_NODE:F_NODE + 1, :], 1.0)
            nc.vector.memset(lhs_b[F_NODE:F_NODE + 1, :], 1.0)
            nc.vector.memset(hT_own[:], 0.0)

            def bn_from_stats(nf, wb, cnt):
                """stats rows [:nf]: col0 sum, col1 sumsq -> col4 sw, col5 sh.
                wb: [nf, 2] AP with (w, b)."""
                s = stats[:nf, :]
                nc.vector.tensor_scalar(out=s[:, 2:3], in0=s[:, 0:1],
                                        scalar1=1.0 / cnt, scalar2=None,
                                        op0=ALU.mult)
                nc.vector.tensor_scalar(out=s[:, 3:4], in0=s[:, 1:2],
                                        scalar1=1.0 / cnt, scalar2=None,
                                        op0=ALU.mult)
                sq = wp.tile([H, 1], FP, tag="bnsq")
                nc.scalar.activation(out=sq[:nf, :], in_=s[:, 2:3],
                                     func=ACTF.Square)
                nc.vector.tensor_tensor(out=s[:, 3:4], in0=s[:, 3:4],
                                        in1=sq[:nf, :], op=ALU.subtract)
                nc.scalar.activation(out=s[:, 3:4], in_=s[:, 3:4],
                                     func=ACTF.Sqrt, bias=epst[:nf, :])
                nc.vector.reciprocal(out=s[:, 3:4], in_=s[:, 3:4])
                nc.vector.tensor_tensor(out=s[:, 4:5], in0=wb[:, 0:1],
                                        in1=s[:, 3:4], op=ALU.mult)
                nc.vector.tensor_tensor(out=s[:, 5:6], in0=s[:, 2:3],
                                        in1=s[:, 4:5], op=ALU.mult)
                nc.vector.tensor_tensor(out=s[:, 5:6], in0=wb[:, 1:2],
                                        in1=s[:, 5:6], op=ALU.subtract)

            def stats_from_hT_ag():
                acc = wp.tile([H, 2], FP, tag="sacc")
                for r in range(N_CORES):
                    slab = wp.tile([H, n_pad], BF, tag="slab")
                    nc.sync.dma_start(out=slab[:], in_=hT_agD[r])
                    t1 = wp.tile([H, 1], FP, tag="st1")
                    nc.vector.tensor_reduce(out=t1[:], in_=slab[:], axis=AX.X,
                                            op=ALU.add)
                    nc.vector.tensor_tensor(out=slab[:], in0=slab[:],
                                            in1=slab[:], op=ALU.mult)
                    t2 = wp.tile([H, 1], FP, tag="st2")
                    nc.vector.tensor_reduce(out=t2[:], in_=slab[:], axis=AX.X,
                                            op=ALU.add)
                    if r == 0:
                        nc.vector.tensor_copy(acc[:, 0:1], t1[:])
                        nc.vector.tensor_copy(acc[:, 1:2], t2[:])
                    else:
                        nc.vector.tensor_tensor(out=acc[:, 0:1], in0=acc[:, 0:1],
                                                in1=t1[:], op=ALU.add)
                        nc.vector.tensor_tensor(out=acc[:, 1:2], in0=acc[:, 1:2],
                                                in1=t2[:], op=ALU.add)
                nc.vector.tensor_copy(stats[:, 0:2], acc[:])

            def table_pass(li, own):
                """li=0: x->(h0|U1|U2); li>=1: h->(ht|A|B). 4-chunk batched."""
                nch = NCH_OWN if own else NCH_ALL
                KD = F_NODE if li == 0 else H
                for ch0 in range(0, nch, 4):
                    k = min(4, nch - ch0)
                    lhs = lhs_a if (ch0 // 4) % 2 == 0 else lhs_b
                    if li == 0:
                        xt = wp.tile([F_NODE, 4 * P], FP, tag="xch")
                        nc.sync.dma_start(
                            out=xt[:, :k * P],
                            in_=(xT_own if own else xT)[:, ch0 * P:(ch0 + k) * P])
                        src_ap = xt[:, :k * P]
                    else:
                        if own:
                            src_ap = hT_own[:, ch0 * P:(ch0 + k) * P]
                        else:
                            slab = wp.tile([H, 4 * P], BF, tag="hslab")
                            r = ch0 // NCH_OWN
                            cc = ch0 % NCH_OWN
                            nc.sync.dma_start(
                                out=slab[:, :k * P],
                                in_=hT_agD[r, :, cc * P:(cc + k) * P])
                            src_ap = slab[:, :k * P]
                    nc.vector.tensor_scalar(
                        out=lhs[:KD, :k * P], in0=src_ap,
                        scalar1=stats[:KD, 4:5], scalar2=stats[:KD, 5:6],
                        op0=ALU.mult, op1=ALU.add)
                    if li == 0:
                        r0, r1, r2 = w0[0][:], w0[1][:], w0[2][:]
                    else:
                        r0, r1, r2 = (wly[li - 1][0][:], wly[li - 1][1][:],
                                      wly[li - 1][2][:])
                    for sub in range(k):
                        ch = ch0 + sub
                        lh = lhs[:KD + 1, sub * P:(sub + 1) * P]
                        ps = psp.tile([P, 192], FP, tag="tb", bufs=4)
                        nc.tensor.matmul(out=ps[:, 0:64], lhsT=lh,
                                         rhs=r0[:KD + 1, :], start=True, stop=True)
                        nc.tensor.matmul(out=ps[:, 64:128], lhsT=lh,
                                         rhs=r1[:KD + 1, :], start=True, stop=True)
                        nc.tensor.matmul(out=ps[:, 128:192], lhsT=lh,
                                         rhs=r2[:KD + 1, :], start=True, stop=True)
                        base = ch * P if own else n_pad + ch * P
                        if li == 0:
                            u1t = wp.tile([P, H], FP, tag="u1t")
                            nc.vector.tensor_copy(u1t[:], ps[:, 64:128])
                            nc.sync.dma_start(out=U1[base:base + P, :], in_=u1t[:])
                            if own:
                                nc.vector.tensor_copy(B_sb[:, ch, :],
                                                      ps[:, 128:192])
                            h1t = wp.tile([P, H], FP, tag="h1t")
                            nc.scalar.activation(out=h1t[:], in_=ps[:, 0:64],
                                                 func=ACTF.Relu)
                            nc.vector.tensor_scalar(
                                out=h1t[:], in0=h1t[:],
                                scalar1=(masko if own else maska)[:, ch:ch + 1],
                                scalar2=None, op0=ALU.mult)
                            tp = pst.tile([H, P], FP, tag="tpx")
                            nc.tensor.transpose(out=tp[:], in_=h1t[:],
                                                identity=idt[:])
                            if own:
                                nc.vector.tensor_copy(
                                    hT_own[:, ch * P:(ch + 1) * P], tp[:])
                            else:
                                hbf = wp.tile([H, P], BF, tag="hbf")
                                nc.vector.tensor_copy(hbf[:], tp[:])
                                nc.sync.dma_start(
                                    out=hT_agD[ch // NCH_OWN, :,
                                               (ch % NCH_OWN) * P:
                                               (ch % NCH_OWN + 1) * P],
                                    in_=hbf[:])
                        else:
                            t1t = wp.tile([P, 2 * H], FP, tag="t1t")
                            nc.vector.tensor_scalar(
                                out=t1t[:, 0:64], in0=ps[:, 0:64],
                                scalar1=(dinvo if own else dinva)[:, ch:ch + 1],
                                scalar2=None, op0=ALU.mult)
                            nc.vector.tensor_copy(t1t[:, 64:128], ps[:, 64:128])
                            nc.sync.dma_start(out=T1[base:base + P, :], in_=t1t[:])
                            if own:
                                nc.vector.tensor_copy(B_sb[:, ch, :],
                                                      ps[:, 128:192])

            # ================= layer 0 =================
            # x stats
            CW = 2048
            nxc = (NT + CW - 1) // CW
            for ci_ in range(nxc):
                w = min(CW, NT - ci_ * CW)
                xc = wp.tile([F_NODE, CW], FP, tag="xstat")
                nc.sync.dma_start(out=xc[:, :w], in_=xT[:, ci_ * CW:ci_ * CW + w])
                t1 = wp.tile([H, 1], FP, tag="st1")
                nc.vector.tensor_reduce(out=t1[:F_NODE, :], in_=xc[:, :w],
                                        axis=AX.X, op=ALU.add)
                nc.vector.tensor_tensor(out=xc[:, :w], in0=xc[:, :w],
                                        in1=xc[:, :w], op=ALU.mult)
                t2 = wp.tile([H, 1], FP, tag="st2")
                nc.vector.tensor_reduce(out=t2[:F_NODE, :], in_=xc[:, :w],
                                        axis=AX.X, op=ALU.add)
                if ci_ == 0:
                    nc.vector.tensor_copy(stats[:F_NODE, 0:1], t1[:F_NODE, :])
                    nc.vector.tensor_copy(stats[:F_NODE, 1:2], t2[:F_NODE, :])
                else:
                    nc.vector.tensor_tensor(out=stats[:F_NODE, 0:1],
                                            in0=stats[:F_NODE, 0:1],
                                            in1=t1[:F_NODE, :], op=ALU.add)
                    nc.vector.tensor_tensor(out=stats[:F_NODE, 1:2],
                                            in0=stats[:F_NODE, 1:2],
                                            in1=t2[:F_NODE, :], op=ALU.add)
            bn_from_stats(F_NODE, bnx[:], float(N_NODES))
            table_pass(0, True)
            table_pass(0, False)
            nc.sync.dma_start(out=U1[ZR:ZR + 1, :], in_=zrow[0:1, 0:H])

            # layer-0 edge phase: build er and Ce_i (per 8-col block)
            for (D, nb, slot0, cc0) in (gmeta if PH in ("all", "l0e") else []):
                gidx = slot0 // P
                for b_ in range(nb):
                    nreal = min(7, D - b_ * 7)
                    gtb = gp.tile([P, 8, H], FP, tag="gt0")
                    o = (cc0 + b_ * 8) * 8
                    nc.gpsimd.dma_gather(
                        out_ap=gtb[:], in_ap=U1[BIAS16:], idxs_ap=it[:, o:o + 64],
                        num_idxs=1024, num_idxs_reg=1024, elem_size=H,
                        queue_num=b_ % 2)
                    eab = wp.tile([F_EDGE, 8 * P], FP, tag="ea")
                    nc.sync.dma_start(
                        out=eab[:],
                        in_=eaT[:, (cc0 + b_ * 8) * P:(cc0 + b_ * 8 + 8) * P])
                    etb = wp.tile([P, 8, H], FP, tag="et")
                    for jj in range(nreal):
                        pse = pst.tile([P, H], FP, tag="tpx")
                        nc.tensor.matmul(out=pse[:],
                                         lhsT=eab[:, jj * P:(jj + 1) * P],
                                         rhs=wce0[:], start=True, stop=True)
                        nc.vector.tensor_tensor(out=etb[:, jj, :],
                                                in0=gtb[:, jj, :], in1=pse[:],
                                                op=ALU.add)
                    nc.vector.tensor_tensor(
                        out=etb[:, 0:nreal, :], in0=etb[:, 0:nreal, :],
                        in1=B_sb[:, gidx:gidx + 1, :].to_broadcast([P, nreal, H]),
                        op=ALU.add)
                    nc.scalar.activation(out=etb[:, 0:nreal, :],
                                         in_=etb[:, 0:nreal, :], func=ACTF.Relu)
                    ce3b = wp.tile([P, 8, 192], FP, tag="ce3")
                    for jj in range(nreal):
                        tp = pst.tile([H, P], FP, tag="tpx")
                        nc.tensor.transpose(out=tp[:], in_=etb[:, jj, :],
                                            identity=idt[:])
                        erT = wp.tile([H, P], FP, tag="erT")
                        nc.vector.tensor_copy(erT[:], tp[:])
                        psc = psp.tile([P, 192], FP, tag="tb", bufs=4)
                        for i3 in range(3):
                            nc.tensor.matmul(out=psc[:, i3 * 64:(i3 + 1) * 64],
                                             lhsT=erT[:], rhs=wce[i3][:],
                                             start=True, stop=True)
                        nc.vector.tensor_copy(ce3b[:, jj, :], psc[:])
                    for i3 in range(3):
                        nc.sync.dma_start(
                            out=CeD[i3, :, cc0 + b_ * 8:cc0 + b_ * 8 + 8, :],
                            in_=ce3b[:, :, i3 * 64:(i3 + 1) * 64])

            # h1 stats (hT_agD was filled during full pass)
            stats_from_hT_ag()

            # ================= layers 1..3 =================
            gps = psg.tile([H, H], FP, name="gps", tag="hd", padded_shape=[H, 512])   # pooling accum
            for li in range(1, 4):
                bn_from_stats(H, bnly[:, (li - 1) * 2:(li - 1) * 2 + 2],
                              float(N_NODES))
                table_pass(li, True)
                table_pass(li, False)
                nc.sync.dma_start(out=T1[ZR:ZR + 1, :], in_=zrow[0:1, :])

                for gi_, (D, nb, slot0, cc0) in enumerate(
                        gmeta if PH != "lyt" else gmeta[:1]):
                    gidx = slot0 // P
                    hacc = wp.tile([P, H], FP, tag="hacc")
                    for b_ in range(nb):
                        nreal = min(7, D - b_ * 7)
                        gtb = gp.tile([P, 8, 2 * H], FP, tag="gt")
                        o = (cc0 + b_ * 8) * 8
                        nc.gpsimd.dma_gather(
                            out_ap=gtb[:], in_ap=T1[BIAS16:],
                            idxs_ap=it[:, o:o + 64],
                            num_idxs=1024, num_idxs_reg=1024, elem_size=2 * H,
                            queue_num=b_ % 2)
                        ceb = gp.tile([P, 8, H], FP, tag="cet")
                        nc.sync.dma_start(
                            out=ceb[:],
                            in_=CeD[li - 1, :, cc0 + b_ * 8:cc0 + b_ * 8 + 8, :])
                        nc.vector.tensor_tensor(
                            out=ceb[:, 0:nreal, :], in0=ceb[:, 0:nreal, :],
                            in1=gtb[:, 0:nreal, H:2 * H], op=ALU.add)
                        nc.vector.tensor_tensor(
                            out=ceb[:, 0:nreal, :], in0=ceb[:, 0:nreal, :],
                            in1=B_sb[:, gidx:gidx + 1, :].to_broadcast(
                                [P, nreal, H]),
                            op=ALU.add)
                        nc.scalar.activation(out=ceb[:, 0:nreal, :],
                                             in_=ceb[:, 0:nreal, :],
                                             func=ACTF.Sigmoid)
                        nc.vector.tensor_tensor(
                            out=ceb[:, 0:nreal, :], in0=ceb[:, 0:nreal, :],
                            in1=gtb[:, 0:nreal, 0:H], op=ALU.mult)
                        rr = wp.tile([P, H], FP, tag="rr")
                        nc.vector.tensor_reduce(
                            out=rr[:],
                            in_=ceb[:, 0:nreal, :].transpose([0, 2, 1]),
                            axis=AX.X, op=ALU.add)
                        if b_ == 0:
                            nc.vector.tensor_copy(hacc[:], rr[:])
                        else:
                            nc.vector.tensor_tensor(out=hacc[:], in0=hacc[:],
                                                    in1=rr[:], op=ALU.add)
                    nc.scalar.activation(out=hacc[:], in_=hacc[:], func=ACTF.Relu)
                    nc.vector.tensor_scalar(out=hacc[:], in0=hacc[:],
                                            scalar1=dinvo[:, gidx:gidx + 1],
                                            scalar2=None, op0=ALU.mult)
                    if li == 3:
                        gpm = wp.tile([P, H], FP, tag="gpm")
                        nc.vector.tensor_scalar(out=gpm[:], in0=iota64f[:],
                                                scalar1=batv[:, gidx:gidx + 1],
                                                scalar2=None, op0=ALU.is_equal)
                        nc.tensor.matmul(out=gps[:], lhsT=hacc[:], rhs=gpm[:],
                                         start=(gi_ == 0),
                                         stop=(gi_ == len(gmeta) - 1),
                                         skip_group_check=True)
                    else:
                        tp = pst.tile([H, P], FP, tag="tpx")
                        nc.tensor.transpose(out=tp[:], in_=hacc[:],
                                            identity=idt[:])
                        nc.vector.tensor_copy(hT_own[:, slot0:slot0 + P], tp[:])

                if li < 3:
                    hbf = wp.tile([H, n_pad], BF, tag="agbf")
                    nc.vector.tensor_copy(hbf[:], hT_own[:])
                    nc.sync.dma_start(out=ag_in[:], in_=hbf[:])
                    nc.gpsimd.collective_compute(
                        "AllGather", ALU.bypass, replica_groups=RG,
                        ins=[ag_in[:]], outs=[ag_out[:]])
                    for r in range(N_CORES):
                        nc.sync.dma_start(out=hT_agD[r],
                                          in_=ag_out[r * H:(r + 1) * H, :])
                    stats_from_hT_ag()

            # ================= head =================
            gsb = wp.tile([H, H], FP, tag="gsb")
            nc.vector.tensor_copy(gsb[:], gps[:])
            nc.sync.dma_start(out=g_in[:], in_=gsb[:, 0:GPC])
            nc.gpsimd.collective_compute(
                "AllGather", ALU.bypass, replica_groups=RG,
                ins=[g_in[:]], outs=[g_out[:]])
            gT = pp.tile([H + 1, N_GRAPHS], FP)
            nc.vector.memset(gT[H:H + 1, :], 1.0)
            for r in range(N_CORES):
                nc.sync.dma_start(out=gT[:H, r * GPC:(r + 1) * GPC],
                                  in_=g_out[r * H:(r + 1) * H, :])
            # BN over 512 graphs
            nc.vector.tensor_reduce(out=stats[:, 0:1], in_=gT[:H, :], axis=AX.X,
                                    op=ALU.add)
            sq5 = wp.tile([H, N_GRAPHS], FP, tag="sq5")
            nc.scalar.activation(out=sq5[:], in_=gT[:H, :], func=ACTF.Square)
            nc.vector.tensor_reduce(out=stats[:, 1:2], in_=sq5[:], axis=AX.X,
                                    op=ALU.add)
            bn_from_stats(H, bnhd[:, 0:2], float(N_GRAPHS))
            gbn = pp.tile([H + 1, N_GRAPHS], FP)
            nc.vector.memset(gbn[H:H + 1, :], 1.0)
            nc.vector.tensor_scalar(out=gbn[:H, :], in0=gT[:H, :],
                                    scalar1=stats[:, 4:5], scalar2=stats[:, 5:6],
                                    op0=ALU.mult, op1=ALU.add)
            ph = psg.tile([H, N_GRAPHS], FP, name="ph", tag="hd")
            nc.tensor.matmul(out=ph[:], lhsT=whd[0][:], rhs=gbn[:], start=True,
                             stop=True)
            nc.scalar.activation(out=gT[:H, :], in_=ph[:], func=ACTF.Relu)
            nc.vector.tensor_reduce(out=stats[:, 0:1], in_=gT[:H, :], axis=AX.X,
                                    op=ALU.add)
            nc.scalar.activation(out=sq5[:], in_=gT[:H, :], func=ACTF.Square)
            nc.vector.tensor_reduce(out=stats[:, 1:2], in_=sq5[:], axis=AX.X,
                                    op=ALU.add)
            bn_from_stats(H, bnhd[:, 2:4], float(N_GRAPHS))
            nc.vector.tensor_scalar(out=gbn[:H, :], in0=gT[:H, :],
                                    scalar1=stats[:, 4:5], scalar2=stats[:, 5:6],
                                    op0=ALU.mult, op1=ALU.add)
            pl = psg.tile([H, N_GRAPHS], FP, name="pl", tag="hd")
            nc.tensor.matmul(out=pl[:], lhsT=whd[1][:], rhs=gbn[:], start=True,
                             stop=True)
            lgT = pp.tile([H, N_GRAPHS], FP)
            nc.vector.tensor_copy(lgT[:], pl[:])
            # log_softmax per graph: transpose chunks of 128 graphs
            for cg in range(N_GRAPHS // P):
                tp = pst.tile([P, P], FP, tag="tpx")
                nc.tensor.transpose(out=tp[:, 0:N_CLASSES],
                                    in_=lgT[0:N_CLASSES, cg * P:(cg + 1) * P],
                                    identity=idt[0:N_CLASSES, 0:N_CLASSES])
                lg = wp.tile([P, N_CLASSES], FP, tag="lg")
                nc.vector.tensor_copy(lg[:], tp[:, 0:N_CLASSES])
                mx = wp.tile([P, 1], FP, tag="mx")
                nc.vector.tensor_reduce(out=mx[:], in_=lg[:], axis=AX.X,
                                        op=ALU.max)
                nc.vector.tensor_scalar(out=lg[:], in0=lg[:], scalar1=mx[:],
                                        scalar2=None, op0=ALU.subtract)
                ex = wp.tile([P, N_CLASSES], FP, tag="ex")
                sm = wp.tile([P, 1], FP, tag="sm")
                nc.scalar.activation(out=ex[:], in_=lg[:], func=ACTF.Exp,
                                     accum_out=sm[:])
                nc.scalar.activation(out=sm[:], in_=sm[:], func=ACTF.Ln)
                nc.vector.tensor_scalar(out=lg[:], in0=lg[:], scalar1=sm[:],
                                        scalar2=None, op0=ALU.subtract)
                nc.sync.dma_start(out=out[cg * P:(cg + 1) * P, :], in_=lg[:])

    nc.finalize()
    return nc


# --------------------------------------------------------------- run + host --
_CACHE = {}
_RUNNERS = {}


def _get_runner(nc, key):
    """jit-once SPMD runner (no donation, reusable across calls)."""
    if key in _RUNNERS:
        return _RUNNERS[key]
    import jax
    from jax.sharding import Mesh, PartitionSpec
    from jax.experimental.shard_map import shard_map
    from concourse import bass2jax
    bass2jax.install_neuronx_cc_hook()
    partition_name = nc.partition_id_tensor.name if nc.partition_id_tensor else None
    in_names, out_names, out_avals, zero_outs = [], [], [], []
    for alloc in nc.m.functions[0].allocations:
        if not isinstance(alloc, mybir.MemoryLocationSet):
            continue
        name = alloc.memorylocations[0].name
        if alloc.kind == "ExternalInput":
            if name != partition_name:
                in_names.append(name)
        elif alloc.kind == "ExternalOutput":
            out_names.append(name)
            shape = tuple(alloc.tensor_shape)
            dtype = mybir.dt.np(alloc.dtype)
            out_avals.append(jax.core.ShapedArray(shape, dtype))
            zero_outs.append(np.zeros((N_CORES * shape[0],) + shape[1:], dtype))
    all_in = in_names + out_names
    if partition_name is not None:
        all_in = all_in + [partition_name]

    def _body(*args):
        operands = list(args)
        if partition_name is not None:
            operands.append(bass2jax.partition_id_tensor())
        outs = bass2jax._bass_exec_p.bind(
            *operands, out_avals=tuple(out_avals), in_names=tuple(all_in),
            out_names=tuple(out_names), lowering_input_output_aliases=(),
            sim_require_finite=True, sim_require_nnan=True, nc=nc)
        return tuple(outs)

    devices = jax.devices()[:N_CORES]
    mesh = Mesh(np.asarray(devices), ("core",))
    n_ops = len(in_names) + len(zero_outs)
    sharded = jax.jit(
        shard_map(_body, mesh=mesh,
                  in_specs=(PartitionSpec("core"),) * n_ops,
                  out_specs=(PartitionSpec("core"),) * len(out_names),
                  check_rep=False),
        keep_unused=True)

    def run(concat_in):
        return sharded(*[concat_in[n] for n in in_names], *zero_outs)

    _RUNNERS[key] = (run, in_names, out_names)
    return _RUNNERS[key]


def _build_in_maps(inp, meta):
    n_pad, NT, ZR = meta["n_pad"], meta["NT"], meta["ZR"]
    NCH_OWN, NCH_ALL = n_pad // P, NT // P
    n_cc = meta["n_cc"]

    x_pad = np.zeros((NT, F_NODE), np.float32)
    deg_pad = np.zeros(NT, np.float32)
    mask_pad = np.zeros(NT, np.float32)
    for c, ci in enumerate(meta["cores"]):
        b0 = c * n_pad
        sel = ci["x_rows"] >= 0
        ii = np.nonzero(sel)[0]
        x_pad[b0 + ii] = inp["x"][ci["x_rows"][ii]]
        deg_pad[b0:b0 + n_pad] = ci["deg_slot"]
        mask_pad[b0 + ii] = 1.0
    dinv_pad = (1.0 / np.sqrt(np.maximum(deg_pad, 1.0))).astype(np.float32)

    def cols(a, nch):
        return np.ascontiguousarray(a.reshape(nch, P).T).astype(np.float32)

    xT = np.ascontiguousarray(x_pad.T).astype(np.float32)
    dinv_all = cols(dinv_pad, NCH_ALL)
    mask_all = cols(mask_pad, NCH_ALL)

    Wn0, bn0 = inp["Wn0"], inp["bn0"]
    We0 = inp["We0"]
    w_l0 = np.stack([
        np.vstack([Wn0, bn0[None, :]]),
        np.vstack([Wn0 @ We0[:H], (bn0 @ We0[:H])[None, :]]),
        np.vstack([Wn0 @ We0[H:2 * H],
                   (bn0 @ We0[H:2 * H] + inp["be0"])[None, :]]),
    ]).astype(np.float32)
    w_ly = np.zeros((3, 3, H + 1, H), np.float32)
    for i in range(3):
        Wn, Wnb, We, Web = inp["Wn"][i], inp["Wnb"][i], inp["We"][i], inp["Web"][i]
        w_ly[i, 0] = np.vstack([Wn, Wnb[None, :]])
        w_ly[i, 1] = np.vstack([Wn @ We[:H], (Wnb @ We[:H])[None, :]])
        w_ly[i, 2] = np.vstack([Wn @ We[H:2 * H],
                                (Wnb @ We[H:2 * H] + Web)[None, :]])
    w_ce0 = We0[2 * H:].astype(np.float32)
    w_ce = np.stack([inp["We"][i, 2 * H:] for i in range(3)]).astype(np.float32)
    w_hd = np.zeros((2, H + 1, H), np.float32)
    w_hd[0] = np.vstack([inp["Wfc"], inp["bfc"][None, :]])
    w_hd[1, :H, :N_CLASSES] = inp["Wcls"]
    w_hd[1, H, :N_CLASSES] = inp["bcls"]
    bn_ly = np.zeros((H, 8), np.float32)
    for i in range(3):
        bn_ly[:, 2 * i] = inp["bns_w"][i]
        bn_ly[:, 2 * i + 1] = inp["bns_b"][i]
    bn_x = np.stack([inp["bn_feat_w"], inp["bn_feat_b"]], 1).astype(np.float32)
    bn_hd = np.stack([inp["bn_fc_w"], inp["bn_fc_b"],
                      inp["bn_hid_w"], inp["bn_hid_b"]], 1).astype(np.float32)

    in_maps = []
    for c, ci in enumerate(meta["cores"]):
        b0 = c * n_pad
        x_own_pad = x_pad[b0:b0 + n_pad]
        ea_s = np.zeros((n_cc * P, F_EDGE), np.float32)
        se = ci["stream_eid"]
        v = se >= 0
        ea_s[v] = inp["edge_attr"][se[v]]
        in_maps.append(dict(
            xT_own=np.ascontiguousarray(x_own_pad.T).astype(np.float32),
            xT=xT,
            idxs=wrap_idx16(ci["tidx"]),
            eaT=np.ascontiguousarray(ea_s.T).astype(np.float32),
            dinv_own=cols(dinv_pad[b0:b0 + n_pad], NCH_OWN),
            dinv_all=dinv_all,
            mask_own=cols(mask_pad[b0:b0 + n_pad], NCH_OWN),
            mask_all=mask_all,
            bat=cols(ci["bat_slot"], NCH_OWN),
            w_l0=w_l0, w_ly=w_ly, w_ce0=w_ce0, w_ce=w_ce, w_hd=w_hd,
            bn_ly=bn_ly, bn_x=bn_x, bn_hd=bn_hd,
        ))
    return in_maps


def kernel(**inputs):
    inp = {k: np.asarray(v) for k, v in inputs.items()}
    meta = prep(inp["edge_index"], inp["batch"])
    import os
    key = (meta["n_pad"], meta["NT"], meta["n_cc"], tuple(meta["tmpl"]),
           os.environ.get("K_PHASES", "all"))
    if key not in _CACHE:
        _CACHE[key] = build_nc(meta)
    nc = _CACHE[key]
    in_maps = _build_in_maps(inp, meta)
    run, in_names, out_names = _get_runner(nc, key)
    concat_in = {n: np.concatenate([np.asarray(m[n]) for m in in_maps], axis=0)
                 for n in in_names}
    outs = run(concat_in)
    oi = out_names.index("out")
    return np.asarray(outs[oi][:N_GRAPHS], np.float32)


def prepare_timed(**inputs):
    """Build everything once; return a zero-host-cost repeat runner."""
    import jax
    out = kernel(**inputs)  # ensures cache + jit built
    inp = {k: np.asarray(v) for k, v in inputs.items()}
    meta = prep(inp["edge_index"], inp["batch"])
    import os
    key = (meta["n_pad"], meta["NT"], meta["n_cc"], tuple(meta["tmpl"]),
           os.environ.get("K_PHASES", "all"))
    nc = _CACHE[key]
    run, in_names, out_names = _get_runner(nc, key)
    in_maps = _build_in_maps(inp, meta)
    concat_in = {n: jax.device_put(
        np.concatenate([np.asarray(m[n]) for m in in_maps], axis=0))
        for n in in_names}

    def timed_run():
        outs = run(concat_in)
        jax.block_until_ready(outs)
        return outs

    return timed_run, out
